# revision 1
# baseline (speedup 1.0000x reference)
"""DimeNet-style GNN message passing on 8 Trainium2 NeuronCores.

Sharding: edges are packed into 128-edge "windows" such that each window's
triplet count <= K_FIX*128; windows are dealt to 8 cores (graph-parallel).
Each core owns its edges AND all triplets targeting them (gather and
scatter in the interaction block both use idx_kj, so triplet work is fully
local to the owning core).  Gather (x_kj[idx_kj]) is an expand-matmul with
a one-hot matrix; scatter-add is a matmul with the transposed one-hot,
accumulated in PSUM per window.  The only cross-core communication is one
ReduceScatter of the [H, N] atom-message partial sums.
"""
import os
import sys
import numpy as np

sys.path.insert(0, "/opt/trn_rl_repo")

H = 128
NR = 16
NS = 6
L = 2
CUTOFF = 8.0
NCORES = 8
TWO_PI = float(2 * np.pi)
F32 = np.float32
LAST_RESULTS = None


# ----------------------------------------------------------------------------
# host-side helpers
# ----------------------------------------------------------------------------

def _envelope(x):
    x5 = x ** 5
    return np.where(x < 1.0, 1.0 / x - 28.0 * x5 + 48.0 * x5 * x - 21.0 * x5 * x * x, 0.0)


def _pack_edges(deg, n_windows):
    """Deal edges (sorted by degree desc) snake-wise into n_windows windows.
    Returns list of edge-id lists. Balances both edge count and triplet sum."""
    order = np.argsort(-deg, kind="stable")
    wins = [[] for _ in range(n_windows)]
    i = 0
    fwd = True
    for e in order:
        w = i if fwd else n_windows - 1 - i
        wins[w].append(int(e))
        i += 1
        if i == n_windows:
            i = 0
            fwd = not fwd
    return wins


def kernel(**inputs):
    import concourse.bass as bass
    import concourse.bacc as bacc
    import concourse.mybir as mybir
    import concourse.tile as tile
    from concourse.bass import IndirectOffsetOnAxis
    from concourse.bass_utils import run_bass_kernel_spmd

    DT = mybir.dt.float32

    af = np.asarray(inputs["atom_feature"], F32)     # [N,133]
    ef = np.asarray(inputs["edge_feature"], F32)     # [E,14]
    dist = np.asarray(inputs["dist"], F32)           # [E]
    angle = np.asarray(inputs["angle"], F32)         # [T]
    i_idx = np.asarray(inputs["i"]).astype(np.int64)
    j_idx = np.asarray(inputs["j"]).astype(np.int64)
    idx_kj = np.asarray(inputs["idx_kj"]).astype(np.int64)
    ib_eid = np.asarray(inputs["incomebond_edge_ids"]).astype(np.int64)
    ib_atom = np.asarray(inputs["incomebond_index_to_atom"]).astype(np.int64)

    N, FA = af.shape
    E = ef.shape[0]
    T = angle.shape[0]
    FE = ef.shape[1]
    FI = FA + FE                                     # 147

    # --- host precompute (index decode / input gathers / tiny per-edge scalars)
    atom_type = np.argmax(af[:, :100], axis=1)
    x_emb = np.asarray(inputs["emb_table"], F32)[atom_type]          # [N,H]
    d_edge = (dist / CUTOFF).astype(F32)                             # [E]
    env_edge = _envelope(d_edge.astype(np.float64)).astype(F32)      # [E]

    # --- edge -> window packing
    deg = np.bincount(idx_kj, minlength=E)
    # windows total: multiple of 32 (so NW per core is a multiple of 4) and
    # enough that the average triplets/window leaves headroom under 512
    NW_TOT = -(-(-(-E // 128)) // 32) * 32
    while T / NW_TOT > 490.0:
        NW_TOT += 32
    wins = _pack_edges(deg, NW_TOT)
    tmax = max(int(deg[w].sum()) for w in wins if w)
    K_FIX = max(1, -(-tmax // 128))
    TPW = 128 * K_FIX                                # triplet slots per window
    NW = NW_TOT // NCORES                            # windows per core
    EC = NW * 128                                    # edge slots per core
    NSC = -(-NW // 4)                                # superchunks of 4 windows
    assert NSC * 4 == NW, (NW,)

    # deal windows to cores (snake by triplet load)
    wloads = np.array([int(deg[w].sum()) for w in wins])
    worder = np.argsort(-wloads, kind="stable")
    core_wins = [[] for _ in range(NCORES)]
    i = 0
    fwd = True
    for w in worder:
        c = i if fwd else NCORES - 1 - i
        core_wins[c].append(int(w))
        i += 1
        if i == NCORES:
            i = 0
            fwd = not fwd

    # triplets grouped by target edge
    t_order = np.argsort(idx_kj, kind="stable")
    t_sorted_edge = idx_kj[t_order]
    seg_starts = np.searchsorted(t_sorted_edge, np.arange(E))
    seg_ends = np.searchsorted(t_sorted_edge, np.arange(E), side="right")

    owner = np.full(E, -1, np.int32)
    localrow = np.full(E, -1, np.int32)

    per_core = []
    for c in range(NCORES):
        edge_ids = np.full(EC, -1, np.int64)
        for wl, w in enumerate(core_wins[c]):
            es = wins[w]
            edge_ids[wl * 128: wl * 128 + len(es)] = es
        real = edge_ids >= 0
        re = edge_ids[real]
        owner[re] = c
        localrow[re] = np.nonzero(real)[0].astype(np.int32)

        # per-edge device inputs (feature-major, padded edges -> 0)
        ibT = np.zeros((FI, EC), F32)
        embiT = np.zeros((H, EC), F32)
        embjT = np.zeros((H, EC), F32)
        dE = np.full((1, EC), 0.5, F32)
        envE = np.zeros((1, EC), F32)
        ibT[:FA, real] = af[j_idx[re]].T
        ibT[FA:, real] = ef[re].T
        embiT[:, real] = x_emb[i_idx[re]].T
        embjT[:, real] = x_emb[j_idx[re]].T
        dE[0, real] = d_edge[re]
        envE[0, real] = env_edge[re]

        # triplet slots
        TP = NW * TPW
        tripmeta = np.zeros((4, TP), F32)            # angle, dkj, envkj, segrel
        tripmeta[1] = 0.5
        tripmeta[3] = -1.0
        for wl in range(NW):
            pos = wl * TPW
            for p in range(128):
                e = edge_ids[wl * 128 + p]
                if e < 0:
                    continue
                ts = t_order[seg_starts[e]:seg_ends[e]]
                n = len(ts)
                if n == 0:
                    continue
                tripmeta[0, pos:pos + n] = angle[ts]
                tripmeta[1, pos:pos + n] = d_edge[e]
                tripmeta[2, pos:pos + n] = env_edge[e]
                tripmeta[3, pos:pos + n] = float(p)
                pos += n
            assert pos <= (wl + 1) * TPW
        # segrel transposed into columns of 128 for the scatter one-hot
        segcolT = np.ascontiguousarray(
            tripmeta[3].reshape(NW * K_FIX, 128).T)   # [128, NW*K_FIX]
        # one row per window: [angle | dkj | envkj | segrel] concatenated
        tripcat = np.ascontiguousarray(
            tripmeta.reshape(4, NW, TPW).transpose(1, 0, 2).reshape(1, NW * 4 * TPW))
        per_core.append(dict(ibT=ibT, embiT=embiT, embjT=embjT, dE=dE, envE=envE,
                             tripcat=tripcat, segcolT=segcolT,
                             edge_ids=edge_ids))

    # --- income bonds -> owner of source edge, laid out by target-atom window
    # atom windows: multiple of 32 so each core's final shard is a multiple of 512
    NAW = -(-(-(-N // 128)) // 32) * 32
    NA = NAW * 128                                   # padded atom count
    ASH = NA // NCORES                               # atoms per core for final
    bond_owner = owner[ib_eid]
    counts = np.zeros((NCORES, NAW), np.int64)
    for c in range(NCORES):
        sel = np.nonzero(bond_owner == c)[0]
        w_of = ib_atom[sel] // 128
        cnt = np.bincount(w_of, minlength=NAW)
        counts[c] = cnt
    K_A = max(1, -(-int(counts.max()) // 128))
    BPW = 128 * K_A
    BP = NAW * BPW
    for c in range(NCORES):
        srwar = np.zeros((1, BP), np.int32)
        tgw = np.full((1, BP), -1.0, F32)
        sel = np.nonzero(bond_owner == c)[0]
        aw = ib_atom[sel] // 128
        order2 = np.argsort(aw, kind="stable")
        sel = sel[order2]
        aw = aw[order2]
        starts = np.searchsorted(aw, np.arange(NAW))
        ends = np.searchsorted(aw, np.arange(NAW), side="right")
        for w in range(NAW):
            b = sel[starts[w]:ends[w]]
            n = len(b)
            srwar[0, w * BPW: w * BPW + n] = localrow[ib_eid[b]]
            tgw[0, w * BPW: w * BPW + n] = (ib_atom[b] - 128 * w).astype(F32)
        per_core[c]["srcrow"] = np.ascontiguousarray(srwar.reshape(NAW * K_A, 128).T)
        per_core[c]["tgtrel"] = np.ascontiguousarray(tgw.reshape(NAW * K_A, 128).T)
        afT = np.zeros((FA, ASH), F32)
        lo = c * ASH
        hi = min(N, lo + ASH)
        if hi > lo:
            afT[:, :hi - lo] = af[lo:hi].T
        per_core[c]["afT"] = afT

    # --- replicated weights / constants
    W = {k: np.asarray(v, F32) for k, v in inputs.items()
         if k not in ("atom_feature", "edge_feature", "dist", "angle", "i", "j",
                      "idx_kj", "idx_ji", "incomebond_edge_ids",
                      "incomebond_index_to_atom")}
    bf = W["bessel_freq"]                            # [NR] = pi*(1..NR)
    const = dict(
        ones512=np.ones((1, 512), F32),
        zeros512=np.zeros((128, 512), F32),
        q025=np.full((1, NS), 0.25, F32),
        svecn=(np.arange(NS, dtype=F32) / TWO_PI).reshape(1, NS),
        freqn=(bf / TWO_PI).reshape(1, NR).astype(F32),
        iota_mat=np.tile(np.arange(128, dtype=F32), (128, 1)),
        iota_col=np.arange(128, dtype=F32).reshape(128, 1),
        identity=np.eye(128, dtype=F32),
        Wi1a=W["W_i1_w"][:128], Wi1b=W["W_i1_w"][128:FI],
        b_i1=W["W_i1_b"].reshape(H, 1),
        Wrbf=W["lin_rbf_w"], b_rbf=W["lin_rbf_b"].reshape(H, 1),
        Wemb_i=W["lin_emb_w"][:H], Wemb_j=W["lin_emb_w"][H:2 * H],
        Wemb_r=W["lin_emb_w"][2 * H:], b_emb=W["lin_emb_b"].reshape(H, 1),
        Woa1=W["W_o_w"][:128], Woa2=W["W_o_w"][128:FA],
        Wom=W["W_o_w"][FA:], b_o=W["W_o_b"].reshape(H, 1),
        # REP6[r, s*16+r'] = delta(r,r');  REPC[s', s*16+r] = delta(s,s')
        REP6=np.tile(np.eye(NR, dtype=F32), (1, NS)),
        REPC=np.repeat(np.eye(NS, dtype=F32), NR, axis=1),
    )
    for l in range(L):
        const[f"Wkj{l}"] = W["L_kj_w"][l]
        const[f"b_kj{l}"] = W["L_kj_b"][l].reshape(H, 1)
        const[f"Wrbf2{l}"] = W["L_rbf2_w"][l]
        const[f"b_rbf2r{l}"] = W["L_rbf2_b"][l].reshape(1, H)
        const[f"Wsbf1{l}"] = W["L_sbf1_w"][l]
        const[f"Wsbf2{l}"] = W["L_sbf2_w"][l]
        const[f"Wdown{l}"] = W["L_down_w"][l]
        const[f"bdownr{l}"] = W["L_down_b"][l].reshape(1, H)
        const[f"Wup{l}"] = W["L_up_w"][l]
        const[f"bupr{l}"] = np.tile(W["L_up_b"][l].reshape(1, H), (1, K_FIX))
        const[f"Wres1_{l}"] = W["L_res1_w"][l]
        const[f"b_res1_{l}"] = W["L_res1_b"][l].reshape(H, 1)
        const[f"Wres2_{l}"] = W["L_res2_w"][l]
        const[f"b_res2_{l}"] = W["L_res2_b"][l].reshape(H, 1)

    # ------------------------------------------------------------------
    # build the Bass program (identical for all cores)
    # ------------------------------------------------------------------
    nc = bacc.Bacc("TRN2", target_bir_lowering=False, debug=False,
                   num_devices=NCORES)

    def din(name, arr):
        return nc.dram_tensor(name, list(arr.shape), DT if arr.dtype == F32
                              else mybir.dt.int32, kind="ExternalInput")

    d_const = {k: din(k, v) for k, v in const.items()}
    p0 = per_core[0]
    d_ibT = din("ibT", p0["ibT"])
    d_embiT = din("embiT", p0["embiT"])
    d_embjT = din("embjT", p0["embjT"])
    d_dE = din("dE", p0["dE"])
    d_envE = din("envE", p0["envE"])
    d_tripcat = din("tripcat", p0["tripcat"])
    d_segcolT = din("segcolT", p0["segcolT"])
    d_srcrow = din("srcrow", p0["srcrow"])
    d_tgtrel = din("tgtrel", p0["tgtrel"])
    d_afT = din("afT", p0["afT"])
    d_out = nc.dram_tensor("outT", [H, ASH], DT, kind="ExternalOutput")

    TP = NW * TPW

    with tile.TileContext(nc) as tc:
        with (
            tc.tile_pool(name="const", bufs=1) as cpool,
            tc.tile_pool(name="sb", bufs=3) as sb,
            tc.tile_pool(name="sbsmall", bufs=3) as sbs,
            tc.tile_pool(name="psb", bufs=3, space="PSUM") as psb,      # [128,512]
            tc.tile_pool(name="pss", bufs=3, space="PSUM") as pss,      # [128,128]
            tc.tile_pool(name="psagg", bufs=2, space="PSUM") as psagg,  # agg
            tc.tile_pool(name="dram", bufs=1, space="DRAM") as dram,
        ):
            C = {}
            for k, v in const.items():
                t = cpool.tile(list(v.shape), DT, tag=k)
                nc.sync.dma_start(t[:], d_const[k][:])
                C[k] = t

            msg = [dram.tile([H, EC], DT, tag="msgA", name="msgA"),
                   dram.tile([H, EC], DT, tag="msgB", name="msgB")]
            rbfeT = dram.tile([H, EC], DT, tag="rbfeT")
            msgRM = dram.tile([EC, H], DT, tag="msgRM")
            apart = dram.tile([NCORES, H, ASH], DT, tag="apart")
            asum = dram.tile([H, ASH], DT, tag="asum")

            RELU = mybir.ActivationFunctionType.Relu
            SIN = mybir.ActivationFunctionType.Sin
            ADD = mybir.AluOpType.add
            MULT = mybir.AluOpType.mult
            ISEQ = mybir.AluOpType.is_equal
            MAX = mybir.AluOpType.max

            def sin_of_psum(p_arg, parts, width, tag):
                """p_arg holds arg/(2pi); returns SBUF tile sin(arg) [parts,width]."""
                qi = sbs.tile([parts, width], mybir.dt.int32, tag="sinqi")
                nc.vector.tensor_copy(qi[:], p_arg[:])
                qf = sbs.tile([parts, width], DT, tag="sinqf")
                nc.vector.tensor_copy(qf[:], qi[:])
                y = sbs.tile([parts, width], DT, tag="siny")
                nc.vector.scalar_tensor_tensor(y[:], qf[:], -1.0, p_arg[:], MULT, ADD)
                s = sbs.tile([parts, width], DT, tag="sins")
                nc.scalar.activation(s[:], y[:], SIN, scale=TWO_PI)
                return s

            # ---------------- phase 0: embedding ----------------
            for sc in range(NSC):
                cs = slice(sc * 512, sc * 512 + 512)
                ib_hi = sb.tile([128, 512], DT, tag="mt")
                nc.sync.dma_start(ib_hi[:], d_ibT[0:128, cs])
                ib_lo = sb.tile([FI - 128, 512], DT, tag="afl")
                nc.sync.dma_start(ib_lo[:], d_ibT[128:FI, cs])
                pm = psb.tile([128, 512], DT, tag="big")
                nc.tensor.matmul(pm[:], C["Wi1a"][:], ib_hi[:], start=True, stop=False)
                nc.tensor.matmul(pm[:], C["Wi1b"][:], ib_lo[:], start=False, stop=True)
                m0 = sb.tile([128, 512], DT, tag="s1")
                nc.vector.scalar_tensor_tensor(m0[:], pm[:], C["b_i1"][:, :1], C["zeros512"][:, :512], ADD, MAX)
                nc.sync.dma_start(msg[0][:, cs], m0[:])

                drow = sbs.tile([1, 512], DT, tag="ang_r")
                nc.sync.dma_start(drow[:], d_dE[:, cs])
                erow = sbs.tile([1, 512], DT, tag="env_r")
                nc.sync.dma_start(erow[:], d_envE[:, cs])
                parg = pss.tile([NR, 512], DT, tag="small")
                nc.tensor.matmul(parg[:], C["freqn"][:], drow[:], start=True, stop=True)
                sin16 = sin_of_psum(parg, NR, 512, "e")
                penv = pss.tile([NR, 512], DT, tag="small")
                nc.tensor.matmul(penv[:], C["ones512"][:, :NR], erow[:], start=True, stop=True)
                rbf0 = sb.tile([NR, 512], DT, tag="rbf16")
                nc.vector.tensor_tensor(rbf0[:], sin16[:], penv[:], op=MULT)
                prh = psb.tile([128, 512], DT, tag="big")
                nc.tensor.matmul(prh[:], C["Wrbf"][:], rbf0[:], start=True, stop=True)
                rbfh = sb.tile([128, 512], DT, tag="s2")
                nc.vector.scalar_tensor_tensor(rbfh[:], prh[:], C["b_rbf"][:, :1], C["zeros512"][:, :512], ADD, MAX)

                embi = sb.tile([128, 512], DT, tag="kj")
                nc.sync.dma_start(embi[:], d_embiT[:, cs])
                embj = sb.tile([128, 512], DT, tag="xkr")
                nc.sync.dma_start(embj[:], d_embjT[:, cs])
                pre = psb.tile([128, 512], DT, tag="big")
                nc.tensor.matmul(pre[:], C["Wemb_i"][:], embi[:], start=True, stop=False)
                nc.tensor.matmul(pre[:], C["Wemb_j"][:], embj[:], start=False, stop=False)
                nc.tensor.matmul(pre[:], C["Wemb_r"][:], rbfh[:], start=False, stop=True)
                rbe = sb.tile([128, 512], DT, tag="mnew")
                nc.vector.scalar_tensor_tensor(rbe[:], pre[:], C["b_emb"][:, :1], C["zeros512"][:, :512], ADD, MAX)
                nc.sync.dma_start(rbfeT[:, cs], rbe[:])

            # ---------------- phase 1: interaction layers ----------------
            for l in range([L, 0][os.environ.get("SKIP_P1") == "1"]):
                src, dst = msg[l % 2], msg[(l + 1) % 2]
                for sc in range(NSC):
                    cs = slice(sc * 512, sc * 512 + 512)
                    mt = sb.tile([128, 512], DT, tag="mt")
                    nc.sync.dma_start(mt[:], src[:, cs])
                    ret = sb.tile([128, 512], DT, tag="ret")
                    nc.sync.dma_start(ret[:], rbfeT[:, cs])
                    pkj = psb.tile([128, 512], DT, tag="big")
                    nc.tensor.matmul(pkj[:], C[f"Wkj{l}"][:], mt[:], start=True, stop=True)
                    kj = sb.tile([128, 512], DT, tag="kj")
                    nc.vector.scalar_tensor_tensor(kj[:], pkj[:], C[f"b_kj{l}"][:, :1], C["zeros512"][:, :512], ADD, MAX)
                    pr = psb.tile([128, 512], DT, tag="big")
                    nc.tensor.matmul(pr[:], C[f"b_rbf2r{l}"][:], C["ones512"][:],
                                     start=True, stop=False)
                    nc.tensor.matmul(pr[:], C[f"Wrbf2{l}"][:], ret[:], start=False, stop=True)
                    xkr = sb.tile([128, 512], DT, tag="xkr")
                    # xkr = relu(pr) * kj
                    nc.vector.scalar_tensor_tensor(xkr[:], pr[:], 0.0, kj[:], MAX, MULT)

                    for wi in range(4):
                        w = 4 * sc + wi
                        ws = slice(wi * 128, wi * 128 + 128)
                        # y = relu(xkr_w @ Wdown + b)   (row-major [e,f])
                        py = pss.tile([128, 128], DT, tag="small")
                        nc.tensor.matmul(py[:], C["ones512"][:, :128], C[f"bdownr{l}"][:],
                                         start=True, stop=False)
                        nc.tensor.matmul(py[:], xkr[:, ws], C[f"Wdown{l}"][:],
                                         start=False, stop=True)
                        y_rm = sb.tile([128, 128], DT, tag="y_rm")
                        nc.vector.tensor_scalar_max(y_rm[:], py[:], 0.0)

                        trow = sbs.tile([1, 4 * TPW], DT, tag="trow")
                        nc.sync.dma_start(trow[:], d_tripcat[:, w * 4 * TPW:(w + 1) * 4 * TPW])
                        ang_r = trow[:, 0:TPW]
                        dkj_r = trow[:, TPW:2 * TPW]
                        env_r = trow[:, 2 * TPW:3 * TPW]
                        seg_r = trow[:, 3 * TPW:4 * TPW]
                        segc = sbs.tile([128, K_FIX], DT, tag="segc")
                        nc.sync.dma_start(segc[:], d_segcolT[:, w * K_FIX:(w + 1) * K_FIX])

                        # sbf for this window: [NS*NR, TPW]
                        pa = pss.tile([NS, TPW], DT, tag="small")
                        nc.tensor.matmul(pa[:], C["q025"][:], C["ones512"][:, :TPW],
                                         start=True, stop=False)
                        nc.tensor.matmul(pa[:], C["svecn"][:], ang_r,
                                         start=False, stop=True)
                        cbf6 = sin_of_psum(pa, NS, TPW, "c")
                        pb = pss.tile([NR, TPW], DT, tag="small")
                        nc.tensor.matmul(pb[:], C["freqn"][:], dkj_r,
                                         start=True, stop=True)
                        sin16 = sin_of_psum(pb, NR, TPW, "t")
                        pe = pss.tile([NR, TPW], DT, tag="small")
                        nc.tensor.matmul(pe[:], C["ones512"][:, :NR], env_r,
                                         start=True, stop=True)
                        rbf16 = sbs.tile([NR, TPW], DT, tag="rbf16")
                        nc.vector.tensor_tensor(rbf16[:], sin16[:], pe[:], op=MULT)
                        pr96 = psb.tile([NS * NR, TPW], DT, tag="big")
                        nc.tensor.matmul(pr96[:], C["REP6"][:], rbf16[:], start=True, stop=True)
                        pc96 = psb.tile([NS * NR, TPW], DT, tag="big")
                        nc.tensor.matmul(pc96[:], C["REPC"][:], cbf6[:], start=True, stop=True)
                        cbf96 = sb.tile([NS * NR, TPW], DT, tag="cbf96")
                        nc.scalar.copy(cbf96[:], pc96[:])
                        sbf = sb.tile([NS * NR, TPW], DT, tag="sbf")
                        nc.vector.tensor_tensor(sbf[:], pr96[:], cbf96[:], op=MULT)

                        ps1 = psb.tile([128, TPW], DT, tag="big")
                        nc.tensor.matmul(ps1[:], C[f"Wsbf1{l}"][:], sbf[:], start=True, stop=True)
                        s1 = sb.tile([128, TPW], DT, tag="s1")
                        nc.vector.tensor_scalar_max(s1[:], ps1[:], 0.0)
                        ps2 = psb.tile([128, TPW], DT, tag="big")
                        nc.tensor.matmul(ps2[:], C[f"Wsbf2{l}"][:], s1[:], start=True, stop=True)
                        s2 = sb.tile([128, TPW], DT, tag="s2")
                        nc.vector.tensor_scalar_max(s2[:], ps2[:], 0.0)

                        # whole-window expand + multiply (N=TPW)
                        segb = sbs.tile([128, TPW], DT, tag="segb", bufs=2)
                        nc.gpsimd.partition_broadcast(segb[:], seg_r)
                        esub = sbs.tile([128, TPW], DT, tag="esub", bufs=2)
                        nc.vector.tensor_scalar(esub[:], segb[:],
                                                C["iota_col"][:, :1], None, ISEQ)
                        px = psb.tile([128, TPW], DT, tag="big")
                        for n0 in range(0, TPW, 512):
                            n1 = min(TPW, n0 + 512)
                            nc.tensor.matmul(px[:, n0:n1], y_rm[:], esub[:, n0:n1],
                                             start=True, stop=True)
                        xs = sb.tile([128, TPW], DT, tag="xs")
                        nc.vector.tensor_tensor(xs[:], px[:], s2[:], op=MULT)
                        # batched up-projection: all K_FIX chunks in one PSUM bank,
                        # one bias seed, one relu evict
                        pz = psb.tile([128, 128 * K_FIX], DT, tag="big")
                        for n0 in range(0, 128 * K_FIX, 512):
                            n1 = min(128 * K_FIX, n0 + 512)
                            nc.tensor.matmul(pz[:, n0:n1], C["ones512"][:, :128],
                                             C[f"bupr{l}"][:, n0:n1],
                                             start=True, stop=False)
                        for k in range(K_FIX):
                            ks = slice(k * 128, k * 128 + 128)
                            nc.tensor.matmul(pz[:, ks], xs[:, ks], C[f"Wup{l}"][:],
                                             start=False, stop=(k == K_FIX - 1))
                        z_rm = sb.tile([128, 128 * K_FIX], DT, tag="z_rm")
                        nc.vector.tensor_scalar_max(z_rm[:], pz[:], 0.0)
                        pagg = psagg.tile([128, 128], DT, tag="agg")
                        for k in range(K_FIX):
                            ks = slice(k * 128, k * 128 + 128)
                            # S_sub[t,e] = (segrel[t] == e)
                            ssub = sbs.tile([128, 128], DT, tag="ssub")
                            nc.vector.tensor_scalar(ssub[:], C["iota_mat"][:],
                                                    segc[:, k:k + 1], None, ISEQ)
                            nc.tensor.matmul(pagg[:], z_rm[:, ks], ssub[:],
                                             start=(k == 0), stop=(k == K_FIX - 1))

                        agg = sb.tile([128, 128], DT, tag="agg")
                        nc.scalar.copy(agg[:], pagg[:])
                        p1 = pss.tile([128, 128], DT, tag="small")
                        nc.tensor.matmul(p1[:], C[f"Wres1_{l}"][:], agg[:], start=True, stop=True)
                        r1 = sbs.tile([128, 128], DT, tag="r1")
                        nc.vector.scalar_tensor_tensor(r1[:], p1[:], C[f"b_res1_{l}"][:, :1], C["zeros512"][:, :128], ADD, MAX)
                        p2 = pss.tile([128, 128], DT, tag="small")
                        nc.tensor.matmul(p2[:], C[f"Wres2_{l}"][:], r1[:], start=True, stop=True)
                        r2 = sbs.tile([128, 128], DT, tag="r2")
                        nc.vector.scalar_tensor_tensor(r2[:], p2[:], C[f"b_res2_{l}"][:, :1], C["zeros512"][:, :128], ADD, MAX)
                        mnew = sb.tile([128, 128], DT, tag="mnew")
                        nc.vector.tensor_tensor(mnew[:], agg[:], r2[:], op=ADD)
                        nc.vector.tensor_tensor(mnew[:], mnew[:], mt[:, ws], op=ADD)
                        nc.sync.dma_start(dst[:, w * 128:(w + 1) * 128], mnew[:])
                        if l == L - 1:
                            pt = pss.tile([128, 128], DT, tag="small")
                            nc.tensor.transpose(pt[:], mnew[:], C["identity"][:])
                            mrm = sbs.tile([128, 128], DT, tag="mrm")
                            nc.scalar.copy(mrm[:], pt[:])
                            nc.sync.dma_start(msgRM[w * 128:(w + 1) * 128, :], mrm[:])

            # ---------------- phase 2: atom aggregation ----------------
            for w in range([NAW, 0][os.environ.get("SKIP_P2") == "1"]):
                pap = psagg.tile([128, 128], DT, tag="agg")
                srt2 = sbs.tile([128, K_A], mybir.dt.int32, tag="srt")
                nc.sync.dma_start(srt2[:], d_srcrow[:, w * K_A:(w + 1) * K_A])
                tgt2 = sbs.tile([128, K_A], DT, tag="tgt")
                nc.sync.dma_start(tgt2[:], d_tgtrel[:, w * K_A:(w + 1) * K_A])
                for k in range(K_A):
                    gath = sbs.tile([128, 128], DT, tag="gath")
                    nc.gpsimd.indirect_dma_start(
                        out=gath[:], out_offset=None,
                        in_=msgRM[:],
                        in_offset=IndirectOffsetOnAxis(ap=srt2[:, k:k + 1], axis=0))
                    sat = sbs.tile([128, 128], DT, tag="sat")
                    nc.vector.tensor_scalar(sat[:], C["iota_mat"][:], tgt2[:, k:k + 1], None, ISEQ)
                    nc.tensor.matmul(pap[:], gath[:], sat[:],
                                     start=(k == 0), stop=(k == K_A - 1))
                apt = sbs.tile([128, 128], DT, tag="apt")
                nc.scalar.copy(apt[:], pap[:])
                blk = w // (NAW // NCORES)
                col = (w % (NAW // NCORES)) * 128
                nc.sync.dma_start(apart[blk, :, col:col + 128], apt[:])

            if os.environ.get("SKIP_COLL") != "1" and os.environ.get("SKIP_P2") != "1":
                nc.gpsimd.collective_compute(
                    "ReduceScatter", ADD,
                    replica_groups=[list(range(NCORES))],
                    ins=[apart.opt()], outs=[asum.opt()])

            # ---------------- phase 3: output ----------------
            for j in range(ASH // 512):
                cs = slice(j * 512, j * 512 + 512)
                afh = sb.tile([128, 512], DT, tag="mt")
                nc.sync.dma_start(afh[:], d_afT[0:128, cs])
                afl = sbs.tile([FA - 128, 512], DT, tag="afl")
                nc.sync.dma_start(afl[:], d_afT[128:FA, cs])
                ams = sb.tile([128, 512], DT, tag="ret")
                nc.sync.dma_start(ams[:], asum[:, cs])
                po = psb.tile([128, 512], DT, tag="big")
                nc.tensor.matmul(po[:], C["Woa1"][:], afh[:], start=True, stop=False)
                nc.tensor.matmul(po[:], C["Woa2"][:], afl[:], start=False, stop=False)
                nc.tensor.matmul(po[:], C["Wom"][:], ams[:], start=False, stop=True)
                ot = sb.tile([128, 512], DT, tag="s1")
                nc.vector.scalar_tensor_tensor(ot[:], po[:], C["b_o"][:, :1], C["zeros512"][:, :512], ADD, MAX)
                nc.sync.dma_start(d_out[:, cs], ot[:])

    nc.compile()

    in_maps = []
    for c in range(NCORES):
        p = per_core[c]
        m = {k: v for k, v in const.items()}
        m.update(ibT=p["ibT"], embiT=p["embiT"], embjT=p["embjT"], dE=p["dE"],
                 envE=p["envE"], tripcat=p["tripcat"], segcolT=p["segcolT"],
                 srcrow=p["srcrow"], tgtrel=p["tgtrel"], afT=p["afT"])
        in_maps.append(m)

    res = run_bass_kernel_spmd(nc, in_maps, core_ids=list(range(NCORES)))
    global LAST_RESULTS
    LAST_RESULTS = res

    out = np.zeros((N, H), F32)
    for c in range(NCORES):
        lo = c * ASH
        hi = min(N, lo + ASH)
        if hi > lo:
            out[lo:hi] = res.results[c]["outT"][:, :hi - lo].T
    return out



# revision 6
# speedup vs baseline: 1.1632x; 1.1632x over previous
"""DimeNet-style GNN message passing on 8 Trainium2 NeuronCores.

Sharding: edges are packed into 128-edge "windows" such that each window's
triplet count <= K_FIX*128; windows are dealt to 8 cores (graph-parallel).
Each core owns its edges AND all triplets targeting them (gather and
scatter in the interaction block both use idx_kj, so triplet work is fully
local to the owning core).  Gather (x_kj[idx_kj]) is an expand-matmul with
a one-hot matrix; scatter-add is a matmul with the transposed one-hot,
accumulated in PSUM per window.  The only cross-core communication is one
ReduceScatter of the [H, N] atom-message partial sums.
"""
import os
import sys
import numpy as np

sys.path.insert(0, "/opt/trn_rl_repo")

H = 128
NR = 16
NS = 6
L = 2
CUTOFF = 8.0
NCORES = 8
TWO_PI = float(2 * np.pi)
F32 = np.float32
LAST_RESULTS = None


# ----------------------------------------------------------------------------
# host-side helpers
# ----------------------------------------------------------------------------

def _envelope(x):
    x5 = x ** 5
    return np.where(x < 1.0, 1.0 / x - 28.0 * x5 + 48.0 * x5 * x - 21.0 * x5 * x * x, 0.0)


def _pack_edges(deg, n_windows):
    """Deal edges (sorted by degree desc) snake-wise into n_windows windows.
    Returns list of edge-id lists. Balances both edge count and triplet sum."""
    order = np.argsort(-deg, kind="stable")
    wins = [[] for _ in range(n_windows)]
    i = 0
    fwd = True
    for e in order:
        w = i if fwd else n_windows - 1 - i
        wins[w].append(int(e))
        i += 1
        if i == n_windows:
            i = 0
            fwd = not fwd
    return wins


def kernel(**inputs):
    import time as _time
    _t0 = _time.time()
    _tick = lambda tag: print(f"[kernel-timing] {tag}: {_time.time() - _t0:.2f}s",
                              file=sys.stderr, flush=True)
    import concourse.bass as bass
    import concourse.bacc as bacc
    import concourse.mybir as mybir
    import concourse.tile as tile
    from concourse.bass import IndirectOffsetOnAxis
    from concourse.bass_utils import run_bass_kernel_spmd

    DT = mybir.dt.float32

    af = np.asarray(inputs["atom_feature"], F32)     # [N,133]
    ef = np.asarray(inputs["edge_feature"], F32)     # [E,14]
    dist = np.asarray(inputs["dist"], F32)           # [E]
    angle = np.asarray(inputs["angle"], F32)         # [T]
    i_idx = np.asarray(inputs["i"]).astype(np.int64)
    j_idx = np.asarray(inputs["j"]).astype(np.int64)
    idx_kj = np.asarray(inputs["idx_kj"]).astype(np.int64)
    ib_eid = np.asarray(inputs["incomebond_edge_ids"]).astype(np.int64)
    ib_atom = np.asarray(inputs["incomebond_index_to_atom"]).astype(np.int64)

    N, FA = af.shape
    E = ef.shape[0]
    T = angle.shape[0]
    FE = ef.shape[1]
    FI = FA + FE                                     # 147

    # --- host precompute (index decode / input gathers / tiny per-edge scalars)
    atom_type = np.argmax(af[:, :100], axis=1)
    x_emb = np.asarray(inputs["emb_table"], F32)[atom_type]          # [N,H]
    d_edge = (dist / CUTOFF).astype(F32)                             # [E]
    env_edge = _envelope(d_edge.astype(np.float64)).astype(F32)      # [E]

    _tick("imports+host-precompute")
    # --- edge -> window packing
    deg = np.bincount(idx_kj, minlength=E)
    # windows total: multiple of 32 (so NW per core is a multiple of 4) and
    # enough that the average triplets/window leaves headroom under 512
    NW_TOT = -(-(-(-E // 128)) // 32) * 32
    while T / NW_TOT > 490.0:
        NW_TOT += 32
    wins = _pack_edges(deg, NW_TOT)
    tmax = max(int(deg[w].sum()) for w in wins if w)
    K_FIX = max(1, -(-tmax // 128))
    TPW = 128 * K_FIX                                # triplet slots per window
    NW = NW_TOT // NCORES                            # windows per core
    EC = NW * 128                                    # edge slots per core
    NSC = -(-NW // 4)                                # superchunks of 4 windows
    assert NSC * 4 == NW, (NW,)

    # deal windows to cores (snake by triplet load)
    wloads = np.array([int(deg[w].sum()) for w in wins])
    worder = np.argsort(-wloads, kind="stable")
    core_wins = [[] for _ in range(NCORES)]
    i = 0
    fwd = True
    for w in worder:
        c = i if fwd else NCORES - 1 - i
        core_wins[c].append(int(w))
        i += 1
        if i == NCORES:
            i = 0
            fwd = not fwd

    # triplets grouped by target edge
    t_order = np.argsort(idx_kj, kind="stable")
    t_sorted_edge = idx_kj[t_order]
    seg_starts = np.searchsorted(t_sorted_edge, np.arange(E))
    seg_ends = np.searchsorted(t_sorted_edge, np.arange(E), side="right")

    owner = np.full(E, -1, np.int32)
    localrow = np.full(E, -1, np.int32)

    per_core = []
    for c in range(NCORES):
        edge_ids = np.full(EC, -1, np.int64)
        for wl, w in enumerate(core_wins[c]):
            es = wins[w]
            edge_ids[wl * 128: wl * 128 + len(es)] = es
        real = edge_ids >= 0
        re = edge_ids[real]
        owner[re] = c
        localrow[re] = np.nonzero(real)[0].astype(np.int32)

        # per-edge device inputs (feature-major, padded edges -> 0)
        ibT = np.zeros((FI, EC), F32)
        embiT = np.zeros((H, EC), F32)
        embjT = np.zeros((H, EC), F32)
        dE = np.full((1, EC), 0.5, F32)
        envE = np.zeros((1, EC), F32)
        ibT[:FA, real] = af[j_idx[re]].T
        ibT[FA:, real] = ef[re].T
        embiT[:, real] = x_emb[i_idx[re]].T
        embjT[:, real] = x_emb[j_idx[re]].T
        dE[0, real] = d_edge[re]
        envE[0, real] = env_edge[re]

        # triplet slots
        TP = NW * TPW
        tripmeta = np.zeros((4, TP), F32)            # angle, dkj, envkj, segrel
        tripmeta[1] = 0.5
        tripmeta[3] = -1.0
        for wl in range(NW):
            pos = wl * TPW
            for p in range(128):
                e = edge_ids[wl * 128 + p]
                if e < 0:
                    continue
                ts = t_order[seg_starts[e]:seg_ends[e]]
                n = len(ts)
                if n == 0:
                    continue
                tripmeta[0, pos:pos + n] = angle[ts]
                tripmeta[1, pos:pos + n] = d_edge[e]
                tripmeta[2, pos:pos + n] = env_edge[e]
                tripmeta[3, pos:pos + n] = float(p)
                pos += n
            assert pos <= (wl + 1) * TPW
        # segrel transposed into columns of 128 for the scatter one-hot
        segcolT = np.ascontiguousarray(
            tripmeta[3].reshape(NW * K_FIX, 128).T)   # [128, NW*K_FIX]
        # one row per window: [angle | dkj | envkj | segrel] concatenated
        tripcat = np.ascontiguousarray(
            tripmeta.reshape(4, NW, TPW).transpose(1, 0, 2).reshape(1, NW * 4 * TPW))
        per_core.append(dict(ibT=ibT, embiT=embiT, embjT=embjT, dE=dE, envE=envE,
                             tripcat=tripcat, segcolT=segcolT,
                             edge_ids=edge_ids))

    # --- income bonds -> owner of source edge, laid out by target-atom window
    # atom windows: multiple of 32 so each core's final shard is a multiple of 512
    NAW = -(-(-(-N // 128)) // 32) * 32
    NA = NAW * 128                                   # padded atom count
    ASH = NA // NCORES                               # atoms per core for final
    bond_owner = owner[ib_eid]
    counts = np.zeros((NCORES, NAW), np.int64)
    for c in range(NCORES):
        sel = np.nonzero(bond_owner == c)[0]
        w_of = ib_atom[sel] // 128
        cnt = np.bincount(w_of, minlength=NAW)
        counts[c] = cnt
    K_A = max(1, -(-int(counts.max()) // 128))
    BPW = 128 * K_A
    BP = NAW * BPW
    for c in range(NCORES):
        srwar = np.zeros((1, BP), np.int32)
        tgw = np.full((1, BP), -1.0, F32)
        sel = np.nonzero(bond_owner == c)[0]
        aw = ib_atom[sel] // 128
        order2 = np.argsort(aw, kind="stable")
        sel = sel[order2]
        aw = aw[order2]
        starts = np.searchsorted(aw, np.arange(NAW))
        ends = np.searchsorted(aw, np.arange(NAW), side="right")
        for w in range(NAW):
            b = sel[starts[w]:ends[w]]
            n = len(b)
            srwar[0, w * BPW: w * BPW + n] = localrow[ib_eid[b]]
            tgw[0, w * BPW: w * BPW + n] = (ib_atom[b] - 128 * w).astype(F32)
        per_core[c]["srcrow"] = np.ascontiguousarray(srwar.reshape(NAW * K_A, 128).T)
        per_core[c]["tgtrel"] = np.ascontiguousarray(tgw.reshape(NAW * K_A, 128).T)
        afT = np.zeros((FA, ASH), F32)
        lo = c * ASH
        hi = min(N, lo + ASH)
        if hi > lo:
            afT[:, :hi - lo] = af[lo:hi].T
        per_core[c]["afT"] = afT

    _tick("per-core packing done")
    # --- replicated weights / constants
    W = {k: np.asarray(v, F32) for k, v in inputs.items()
         if k not in ("atom_feature", "edge_feature", "dist", "angle", "i", "j",
                      "idx_kj", "idx_ji", "incomebond_edge_ids",
                      "incomebond_index_to_atom")}
    bf = W["bessel_freq"]                            # [NR] = pi*(1..NR)
    const = dict(
        ones512=np.ones((1, 512), F32),
        zeros512=np.zeros((128, 512), F32),
        q025=np.full((1, NS), 0.25, F32),
        svecn=(np.arange(NS, dtype=F32) / TWO_PI).reshape(1, NS),
        freqn=(bf / TWO_PI).reshape(1, NR).astype(F32),
        iota_mat=np.tile(np.arange(128, dtype=F32), (128, 1)),
        iota_col=np.arange(128, dtype=F32).reshape(128, 1),
        identity=np.eye(128, dtype=F32),
        Wi1a=W["W_i1_w"][:128], Wi1b=W["W_i1_w"][128:FI],
        b_i1=W["W_i1_b"].reshape(H, 1),
        Wrbf=W["lin_rbf_w"], b_rbf=W["lin_rbf_b"].reshape(H, 1),
        Wemb_i=W["lin_emb_w"][:H], Wemb_j=W["lin_emb_w"][H:2 * H],
        Wemb_r=W["lin_emb_w"][2 * H:], b_emb=W["lin_emb_b"].reshape(H, 1),
        Woa1=W["W_o_w"][:128], Woa2=W["W_o_w"][128:FA],
        Wom=W["W_o_w"][FA:], b_o=W["W_o_b"].reshape(H, 1),
        # REP6[r, s*16+r'] = delta(r,r');  REPC[s', s*16+r] = delta(s,s')
        REP6=np.tile(np.eye(NR, dtype=F32), (1, NS)),
        REPC=np.repeat(np.eye(NS, dtype=F32), NR, axis=1),
    )
    for l in range(L):
        const[f"Wkj{l}"] = W["L_kj_w"][l]
        const[f"b_kj{l}"] = W["L_kj_b"][l].reshape(H, 1)
        const[f"Wrbf2{l}"] = W["L_rbf2_w"][l]
        const[f"b_rbf2r{l}"] = W["L_rbf2_b"][l].reshape(1, H)
        const[f"Wsbf1{l}"] = W["L_sbf1_w"][l]
        const[f"Wsbf2{l}"] = W["L_sbf2_w"][l]
        const[f"Wdown{l}"] = W["L_down_w"][l]
        const[f"bdownr{l}"] = W["L_down_b"][l].reshape(1, H)
        const[f"Wup{l}"] = W["L_up_w"][l]
        const[f"bupr{l}"] = np.tile(W["L_up_b"][l].reshape(1, H), (1, K_FIX))
        const[f"Wres1_{l}"] = W["L_res1_w"][l]
        const[f"b_res1_{l}"] = W["L_res1_b"][l].reshape(H, 1)
        const[f"Wres2_{l}"] = W["L_res2_w"][l]
        const[f"b_res2_{l}"] = W["L_res2_b"][l].reshape(H, 1)

    # ------------------------------------------------------------------
    # build the Bass program (identical for all cores)
    # ------------------------------------------------------------------
    nc = bacc.Bacc("TRN2", target_bir_lowering=False, debug=False,
                   num_devices=NCORES)

    def din(name, arr):
        return nc.dram_tensor(name, list(arr.shape), DT if arr.dtype == F32
                              else mybir.dt.int32, kind="ExternalInput")

    d_const = {k: din(k, v) for k, v in const.items()}
    p0 = per_core[0]
    d_ibT = din("ibT", p0["ibT"])
    d_embiT = din("embiT", p0["embiT"])
    d_embjT = din("embjT", p0["embjT"])
    d_dE = din("dE", p0["dE"])
    d_envE = din("envE", p0["envE"])
    d_tripcat = din("tripcat", p0["tripcat"])
    d_segcolT = din("segcolT", p0["segcolT"])
    d_srcrow = din("srcrow", p0["srcrow"])
    d_tgtrel = din("tgtrel", p0["tgtrel"])
    d_afT = din("afT", p0["afT"])
    d_out = nc.dram_tensor("outT", [H, ASH], DT, kind="ExternalOutput")

    TP = NW * TPW

    with tile.TileContext(nc) as tc:
        with (
            tc.tile_pool(name="const", bufs=1) as cpool,
            tc.tile_pool(name="sb", bufs=3) as sb,
            tc.tile_pool(name="sbsmall", bufs=3) as sbs,
            tc.tile_pool(name="psb", bufs=3, space="PSUM") as psb,      # [128,512]
            tc.tile_pool(name="pss", bufs=3, space="PSUM") as pss,      # [128,128]
            tc.tile_pool(name="psagg", bufs=2, space="PSUM") as psagg,  # agg
            tc.tile_pool(name="dram", bufs=1, space="DRAM") as dram,
        ):
            C = {}
            for k, v in const.items():
                t = cpool.tile(list(v.shape), DT, tag=k)
                nc.sync.dma_start(t[:], d_const[k][:])
                C[k] = t

            msg = [dram.tile([H, EC], DT, tag="msgA", name="msgA"),
                   dram.tile([H, EC], DT, tag="msgB", name="msgB")]
            rbfeT = dram.tile([H, EC], DT, tag="rbfeT")
            msgRM = dram.tile([EC, H], DT, tag="msgRM")
            apart = dram.tile([NCORES, H, ASH], DT, tag="apart")
            asum = dram.tile([H, ASH], DT, tag="asum")

            RELU = mybir.ActivationFunctionType.Relu
            SIN = mybir.ActivationFunctionType.Sin
            ADD = mybir.AluOpType.add
            MULT = mybir.AluOpType.mult
            ISEQ = mybir.AluOpType.is_equal
            MAX = mybir.AluOpType.max

            def sin_of_psum(p_arg, parts, width, tag):
                """p_arg holds arg/(2pi); returns SBUF tile sin(arg) [parts,width]."""
                qi = sbs.tile([parts, width], mybir.dt.int32, tag="sinqi")
                nc.vector.tensor_copy(qi[:], p_arg[:])
                qf = sbs.tile([parts, width], DT, tag="sinqf")
                nc.vector.tensor_copy(qf[:], qi[:])
                y = sbs.tile([parts, width], DT, tag="siny")
                nc.vector.scalar_tensor_tensor(y[:], qf[:], -1.0, p_arg[:], MULT, ADD)
                s = sbs.tile([parts, width], DT, tag="sins")
                nc.scalar.activation(s[:], y[:], SIN, scale=TWO_PI)
                return s

            # ---------------- phase 0: embedding ----------------
            for sc in range(NSC):
                cs = slice(sc * 512, sc * 512 + 512)
                ib_hi = sb.tile([128, 512], DT, tag="mt")
                nc.sync.dma_start(ib_hi[:], d_ibT[0:128, cs])
                ib_lo = sb.tile([FI - 128, 512], DT, tag="afl")
                nc.sync.dma_start(ib_lo[:], d_ibT[128:FI, cs])
                pm = psb.tile([128, 512], DT, tag="big")
                nc.tensor.matmul(pm[:], C["Wi1a"][:], ib_hi[:], start=True, stop=False)
                nc.tensor.matmul(pm[:], C["Wi1b"][:], ib_lo[:], start=False, stop=True)
                m0 = sb.tile([128, 512], DT, tag="s1")
                nc.vector.scalar_tensor_tensor(m0[:], pm[:], C["b_i1"][:, :1], C["zeros512"][:, :512], ADD, MAX)
                nc.sync.dma_start(msg[0][:, cs], m0[:])

                drow = sbs.tile([1, 512], DT, tag="ang_r")
                nc.sync.dma_start(drow[:], d_dE[:, cs])
                erow = sbs.tile([1, 512], DT, tag="env_r")
                nc.sync.dma_start(erow[:], d_envE[:, cs])
                parg = pss.tile([NR, 512], DT, tag="small")
                nc.tensor.matmul(parg[:], C["freqn"][:], drow[:], start=True, stop=True)
                sin16 = sin_of_psum(parg, NR, 512, "e")
                penv = pss.tile([NR, 512], DT, tag="small")
                nc.tensor.matmul(penv[:], C["ones512"][:, :NR], erow[:], start=True, stop=True)
                rbf0 = sb.tile([NR, 512], DT, tag="rbf16")
                nc.vector.tensor_tensor(rbf0[:], sin16[:], penv[:], op=MULT)
                prh = psb.tile([128, 512], DT, tag="big")
                nc.tensor.matmul(prh[:], C["Wrbf"][:], rbf0[:], start=True, stop=True)
                rbfh = sb.tile([128, 512], DT, tag="s2")
                nc.vector.scalar_tensor_tensor(rbfh[:], prh[:], C["b_rbf"][:, :1], C["zeros512"][:, :512], ADD, MAX)

                embi = sb.tile([128, 512], DT, tag="kj")
                nc.sync.dma_start(embi[:], d_embiT[:, cs])
                embj = sb.tile([128, 512], DT, tag="xkr")
                nc.sync.dma_start(embj[:], d_embjT[:, cs])
                pre = psb.tile([128, 512], DT, tag="big")
                nc.tensor.matmul(pre[:], C["Wemb_i"][:], embi[:], start=True, stop=False)
                nc.tensor.matmul(pre[:], C["Wemb_j"][:], embj[:], start=False, stop=False)
                nc.tensor.matmul(pre[:], C["Wemb_r"][:], rbfh[:], start=False, stop=True)
                rbe = sb.tile([128, 512], DT, tag="mnew")
                nc.vector.scalar_tensor_tensor(rbe[:], pre[:], C["b_emb"][:, :1], C["zeros512"][:, :512], ADD, MAX)
                nc.sync.dma_start(rbfeT[:, cs], rbe[:])

            # ---------------- phase 1: interaction layers ----------------
            for l in range([L, 0][os.environ.get("SKIP_P1") == "1"]):
                src, dst = msg[l % 2], msg[(l + 1) % 2]
                for sc in range(NSC):
                    cs = slice(sc * 512, sc * 512 + 512)
                    mt = sb.tile([128, 512], DT, tag="mt")
                    nc.sync.dma_start(mt[:], src[:, cs])
                    ret = sb.tile([128, 512], DT, tag="ret")
                    nc.sync.dma_start(ret[:], rbfeT[:, cs])
                    pkj = psb.tile([128, 512], DT, tag="big")
                    nc.tensor.matmul(pkj[:], C[f"Wkj{l}"][:], mt[:], start=True, stop=True)
                    kj = sb.tile([128, 512], DT, tag="kj")
                    nc.vector.scalar_tensor_tensor(kj[:], pkj[:], C[f"b_kj{l}"][:, :1], C["zeros512"][:, :512], ADD, MAX)
                    pr = psb.tile([128, 512], DT, tag="big")
                    nc.tensor.matmul(pr[:], C[f"b_rbf2r{l}"][:], C["ones512"][:],
                                     start=True, stop=False)
                    nc.tensor.matmul(pr[:], C[f"Wrbf2{l}"][:], ret[:], start=False, stop=True)
                    xkr = sb.tile([128, 512], DT, tag="xkr")
                    # xkr = relu(pr) * kj
                    nc.vector.scalar_tensor_tensor(xkr[:], pr[:], 0.0, kj[:], MAX, MULT)

                    for wi in range(4):
                        w = 4 * sc + wi
                        ws = slice(wi * 128, wi * 128 + 128)
                        # y = relu(xkr_w @ Wdown + b)   (row-major [e,f])
                        py = pss.tile([128, 128], DT, tag="small")
                        nc.tensor.matmul(py[:], C["ones512"][:, :128], C[f"bdownr{l}"][:],
                                         start=True, stop=False)
                        nc.tensor.matmul(py[:], xkr[:, ws], C[f"Wdown{l}"][:],
                                         start=False, stop=True)
                        y_rm = sb.tile([128, 128], DT, tag="y_rm")
                        nc.vector.tensor_scalar_max(y_rm[:], py[:], 0.0)

                        trow = sbs.tile([1, 4 * TPW], DT, tag="trow")
                        nc.sync.dma_start(trow[:], d_tripcat[:, w * 4 * TPW:(w + 1) * 4 * TPW])
                        ang_r = trow[:, 0:TPW]
                        dkj_r = trow[:, TPW:2 * TPW]
                        env_r = trow[:, 2 * TPW:3 * TPW]
                        seg_r = trow[:, 3 * TPW:4 * TPW]
                        segc = sbs.tile([128, K_FIX], DT, tag="segc")
                        nc.sync.dma_start(segc[:], d_segcolT[:, w * K_FIX:(w + 1) * K_FIX])

                        # sbf for this window: [NS*NR, TPW]
                        pa = pss.tile([NS, TPW], DT, tag="small")
                        nc.tensor.matmul(pa[:], C["q025"][:], C["ones512"][:, :TPW],
                                         start=True, stop=False)
                        nc.tensor.matmul(pa[:], C["svecn"][:], ang_r,
                                         start=False, stop=True)
                        cbf6 = sin_of_psum(pa, NS, TPW, "c")
                        pb = pss.tile([NR, TPW], DT, tag="small")
                        nc.tensor.matmul(pb[:], C["freqn"][:], dkj_r,
                                         start=True, stop=True)
                        sin16 = sin_of_psum(pb, NR, TPW, "t")
                        pe = pss.tile([NR, TPW], DT, tag="small")
                        nc.tensor.matmul(pe[:], C["ones512"][:, :NR], env_r,
                                         start=True, stop=True)
                        rbf16 = sbs.tile([NR, TPW], DT, tag="rbf16")
                        nc.vector.tensor_tensor(rbf16[:], sin16[:], pe[:], op=MULT)
                        pr96 = psb.tile([NS * NR, TPW], DT, tag="big")
                        nc.tensor.matmul(pr96[:], C["REP6"][:], rbf16[:], start=True, stop=True)
                        pc96 = psb.tile([NS * NR, TPW], DT, tag="big")
                        nc.tensor.matmul(pc96[:], C["REPC"][:], cbf6[:], start=True, stop=True)
                        cbf96 = sb.tile([NS * NR, TPW], DT, tag="cbf96")
                        nc.scalar.copy(cbf96[:], pc96[:])
                        sbf = sb.tile([NS * NR, TPW], DT, tag="sbf")
                        nc.vector.tensor_tensor(sbf[:], pr96[:], cbf96[:], op=MULT)

                        ps1 = psb.tile([128, TPW], DT, tag="big")
                        nc.tensor.matmul(ps1[:], C[f"Wsbf1{l}"][:], sbf[:], start=True, stop=True)
                        s1 = sb.tile([128, TPW], DT, tag="s1")
                        nc.vector.tensor_scalar_max(s1[:], ps1[:], 0.0)
                        ps2 = psb.tile([128, TPW], DT, tag="big")
                        nc.tensor.matmul(ps2[:], C[f"Wsbf2{l}"][:], s1[:], start=True, stop=True)
                        s2 = sb.tile([128, TPW], DT, tag="s2")
                        nc.vector.tensor_scalar_max(s2[:], ps2[:], 0.0)

                        # whole-window expand + multiply (N=TPW)
                        segb = sbs.tile([128, TPW], DT, tag="segb", bufs=2)
                        nc.gpsimd.partition_broadcast(segb[:], seg_r)
                        esub = sbs.tile([128, TPW], DT, tag="esub", bufs=2)
                        nc.vector.tensor_scalar(esub[:], segb[:],
                                                C["iota_col"][:, :1], None, ISEQ)
                        px = psb.tile([128, TPW], DT, tag="big")
                        for n0 in range(0, TPW, 512):
                            n1 = min(TPW, n0 + 512)
                            nc.tensor.matmul(px[:, n0:n1], y_rm[:], esub[:, n0:n1],
                                             start=True, stop=True)
                        xs = sb.tile([128, TPW], DT, tag="xs")
                        nc.vector.tensor_tensor(xs[:], px[:], s2[:], op=MULT)
                        # batched up-projection: all K_FIX chunks in one PSUM bank,
                        # one bias seed, one relu evict
                        pz = psb.tile([128, 128 * K_FIX], DT, tag="big")
                        for n0 in range(0, 128 * K_FIX, 512):
                            n1 = min(128 * K_FIX, n0 + 512)
                            nc.tensor.matmul(pz[:, n0:n1], C["ones512"][:, :128],
                                             C[f"bupr{l}"][:, n0:n1],
                                             start=True, stop=False)
                        for k in range(K_FIX):
                            ks = slice(k * 128, k * 128 + 128)
                            nc.tensor.matmul(pz[:, ks], xs[:, ks], C[f"Wup{l}"][:],
                                             start=False, stop=(k == K_FIX - 1))
                        z_rm = sb.tile([128, 128 * K_FIX], DT, tag="z_rm")
                        nc.vector.tensor_scalar_max(z_rm[:], pz[:], 0.0)
                        pagg = psagg.tile([128, 128], DT, tag="agg")
                        for k in range(K_FIX):
                            ks = slice(k * 128, k * 128 + 128)
                            # S_sub[t,e] = (segrel[t] == e)
                            ssub = sbs.tile([128, 128], DT, tag="ssub")
                            nc.vector.tensor_scalar(ssub[:], C["iota_mat"][:],
                                                    segc[:, k:k + 1], None, ISEQ)
                            nc.tensor.matmul(pagg[:], z_rm[:, ks], ssub[:],
                                             start=(k == 0), stop=(k == K_FIX - 1))

                        agg = sb.tile([128, 128], DT, tag="agg")
                        nc.scalar.copy(agg[:], pagg[:])
                        p1 = pss.tile([128, 128], DT, tag="small")
                        nc.tensor.matmul(p1[:], C[f"Wres1_{l}"][:], agg[:], start=True, stop=True)
                        r1 = sbs.tile([128, 128], DT, tag="r1")
                        nc.vector.scalar_tensor_tensor(r1[:], p1[:], C[f"b_res1_{l}"][:, :1], C["zeros512"][:, :128], ADD, MAX)
                        p2 = pss.tile([128, 128], DT, tag="small")
                        nc.tensor.matmul(p2[:], C[f"Wres2_{l}"][:], r1[:], start=True, stop=True)
                        r2 = sbs.tile([128, 128], DT, tag="r2")
                        nc.vector.scalar_tensor_tensor(r2[:], p2[:], C[f"b_res2_{l}"][:, :1], C["zeros512"][:, :128], ADD, MAX)
                        mnew = sb.tile([128, 128], DT, tag="mnew")
                        nc.vector.tensor_tensor(mnew[:], agg[:], r2[:], op=ADD)
                        nc.vector.tensor_tensor(mnew[:], mnew[:], mt[:, ws], op=ADD)
                        nc.sync.dma_start(dst[:, w * 128:(w + 1) * 128], mnew[:])
                        if l == L - 1:
                            pt = pss.tile([128, 128], DT, tag="small")
                            nc.tensor.transpose(pt[:], mnew[:], C["identity"][:])
                            mrm = sbs.tile([128, 128], DT, tag="mrm")
                            nc.scalar.copy(mrm[:], pt[:])
                            nc.sync.dma_start(msgRM[w * 128:(w + 1) * 128, :], mrm[:])

            # ---------------- phase 2: atom aggregation ----------------
            for w in range([NAW, 0][os.environ.get("SKIP_P2") == "1"]):
                pap = psagg.tile([128, 128], DT, tag="agg")
                srt2 = sbs.tile([128, K_A], mybir.dt.int32, tag="srt")
                nc.sync.dma_start(srt2[:], d_srcrow[:, w * K_A:(w + 1) * K_A])
                tgt2 = sbs.tile([128, K_A], DT, tag="tgt")
                nc.sync.dma_start(tgt2[:], d_tgtrel[:, w * K_A:(w + 1) * K_A])
                for k in range(K_A):
                    gath = sbs.tile([128, 128], DT, tag="gath")
                    nc.gpsimd.indirect_dma_start(
                        out=gath[:], out_offset=None,
                        in_=msgRM[:],
                        in_offset=IndirectOffsetOnAxis(ap=srt2[:, k:k + 1], axis=0))
                    sat = sbs.tile([128, 128], DT, tag="sat")
                    nc.vector.tensor_scalar(sat[:], C["iota_mat"][:], tgt2[:, k:k + 1], None, ISEQ)
                    nc.tensor.matmul(pap[:], gath[:], sat[:],
                                     start=(k == 0), stop=(k == K_A - 1))
                apt = sbs.tile([128, 128], DT, tag="apt")
                nc.scalar.copy(apt[:], pap[:])
                blk = w // (NAW // NCORES)
                col = (w % (NAW // NCORES)) * 128
                nc.sync.dma_start(apart[blk, :, col:col + 128], apt[:])

            if os.environ.get("SKIP_COLL") != "1" and os.environ.get("SKIP_P2") != "1":
                nc.gpsimd.collective_compute(
                    "ReduceScatter", ADD,
                    replica_groups=[list(range(NCORES))],
                    ins=[apart.opt()], outs=[asum.opt()])

            # ---------------- phase 3: output ----------------
            for j in range(ASH // 512):
                cs = slice(j * 512, j * 512 + 512)
                afh = sb.tile([128, 512], DT, tag="mt")
                nc.sync.dma_start(afh[:], d_afT[0:128, cs])
                afl = sbs.tile([FA - 128, 512], DT, tag="afl")
                nc.sync.dma_start(afl[:], d_afT[128:FA, cs])
                ams = sb.tile([128, 512], DT, tag="ret")
                nc.sync.dma_start(ams[:], asum[:, cs])
                po = psb.tile([128, 512], DT, tag="big")
                nc.tensor.matmul(po[:], C["Woa1"][:], afh[:], start=True, stop=False)
                nc.tensor.matmul(po[:], C["Woa2"][:], afl[:], start=False, stop=False)
                nc.tensor.matmul(po[:], C["Wom"][:], ams[:], start=False, stop=True)
                ot = sb.tile([128, 512], DT, tag="s1")
                nc.vector.scalar_tensor_tensor(ot[:], po[:], C["b_o"][:, :1], C["zeros512"][:, :512], ADD, MAX)
                nc.sync.dma_start(d_out[:, cs], ot[:])

    _tick("bass program built")
    nc.compile()
    _tick("nc.compile done")

    in_maps = []
    for c in range(NCORES):
        p = per_core[c]
        m = {k: v for k, v in const.items()}
        m.update(ibT=p["ibT"], embiT=p["embiT"], embjT=p["embjT"], dE=p["dE"],
                 envE=p["envE"], tripcat=p["tripcat"], segcolT=p["segcolT"],
                 srcrow=p["srcrow"], tgtrel=p["tgtrel"], afT=p["afT"])
        in_maps.append(m)

    res = run_bass_kernel_spmd(nc, in_maps, core_ids=list(range(NCORES)))
    _tick("run_bass_kernel_spmd done")
    global LAST_RESULTS
    LAST_RESULTS = res

    out = np.zeros((N, H), F32)
    for c in range(NCORES):
        lo = c * ASH
        hi = min(N, lo + ASH)
        if hi > lo:
            out[lo:hi] = res.results[c]["outT"][:, :hi - lo].T
    return out



# revision 31
# speedup vs baseline: 7.4333x; 6.3904x over previous
"""DimeNet-style GNN message passing on 8 Trainium2 NeuronCores.

Edges are packed into 128-edge windows (triplet load balanced); windows are
dealt to 8 cores.  All triplet work is local to the owning core (gather and
scatter both use idx_kj).  Gather = one-hot expand matmul, scatter-add =
matmul with the transposed one-hot.  Cross-core communication is a single
ReduceScatter of the [H, N] atom-message partial sums.

Wall-clock-oriented design:
 - inputs shipped as a few large arrays (per-edge atom-type indices expanded
   on device via one-hot matmuls against 100-row tables; per-triplet radial
   basis reconstructed on device from per-edge rbf0) -- ~60MB instead of
   ~260MB over the axon tunnel
 - host packing fully vectorized numpy
 - async upload overlapped with Bass program build + NEFF compile
 - compiled executable serialized to /tmp; warm runs skip build+compile
"""
import os
import sys
import pickle
import hashlib
import tempfile

sys.path.insert(0, "/opt/trn_rl_repo")

import numpy as np

H = 128
NR = 16
NS = 6
L = 2
CUTOFF = 8.0
NCORES = 8
TWO_PI = float(2 * np.pi)
F32 = np.float32
I32 = np.int32
VERSION = "dimenet-fast-v3"
CACHE_DIR = "/tmp/bass_dimenet_cache"
LAST_RESULTS = None

_PROC_MEMO = {}


def _envelope(x):
    x5 = x ** 5
    return np.where(x < 1.0, 1.0 / x - 28.0 * x5 + 48.0 * x5 * x - 21.0 * x5 * x * x, 0.0)


def _ceil_to(x, m):
    return -(-x // m) * m


# ----------------------------------------------------------------------------
# planning + packing (vectorized host code)
# ----------------------------------------------------------------------------

def _plan(E, T, N):
    NW_TOT = _ceil_to(-(-E // 128), 32)
    while T / NW_TOT > 490.0:
        NW_TOT += 32
    NAW = _ceil_to(-(-N // 128), 32)
    return NW_TOT, NAW


def _pack(inputs):
    af = np.asarray(inputs["atom_feature"], F32)     # [N,133]
    ef = np.asarray(inputs["edge_feature"], F32)     # [E,14]
    dist = np.asarray(inputs["dist"], F32)           # [E]
    angle = np.asarray(inputs["angle"], F32)         # [T]
    i_idx = np.asarray(inputs["i"]).astype(np.int64)
    j_idx = np.asarray(inputs["j"]).astype(np.int64)
    idx_kj = np.asarray(inputs["idx_kj"]).astype(np.int64)
    ib_eid = np.asarray(inputs["incomebond_edge_ids"]).astype(np.int64)
    ib_atom = np.asarray(inputs["incomebond_index_to_atom"]).astype(np.int64)

    N, FA = af.shape
    E = ef.shape[0]
    T = angle.shape[0]

    atom_type = np.argmax(af[:, :100], axis=1).astype(F32)
    d_edge = (dist / CUTOFF).astype(F32)
    env_edge = _envelope(d_edge.astype(np.float64)).astype(F32)
    # edge-row layout (base-partition-0 groups): 0:33 af33[j], 33:47 ef,
    # 47 t_i, 48 t_j, 49 dE, 50 envE

    NW_TOT, NAW = _plan(E, T, N)
    deg = np.bincount(idx_kj, minlength=E)

    # snake-deal edges (by triplet count desc) into NW_TOT windows
    order = np.argsort(-deg, kind="stable")
    r = np.arange(E) // NW_TOT
    p = np.arange(E) % NW_TOT
    wsn = np.where(r % 2 == 0, p, NW_TOT - 1 - p)
    window = np.empty(E, np.int64)
    window[order] = wsn
    slot = np.empty(E, np.int64)
    slot[order] = r
    assert slot.max() < 128

    wload = np.bincount(window, weights=deg.astype(np.float64), minlength=NW_TOT).astype(np.int64)
    K_FIX = max(1, -(-int(wload.max()) // 128))
    TPW = 128 * K_FIX

    # snake-deal windows (by load desc) onto cores
    worder = np.argsort(-wload, kind="stable")
    r2 = np.arange(NW_TOT) // NCORES
    p2 = np.arange(NW_TOT) % NCORES
    csn = np.where(r2 % 2 == 0, p2, NCORES - 1 - p2)
    wcore = np.empty(NW_TOT, np.int64)
    wcore[worder] = csn
    wlocal = np.empty(NW_TOT, np.int64)
    wlocal[worder] = r2

    NW = NW_TOT // NCORES
    EC = NW * 128
    NSC = NW // 4
    assert NSC * 4 == NW

    oc = wcore[window]                               # [E] owner core
    lr = wlocal[window] * 128 + slot                 # [E] local row on core

    # --- per-edge rows: [8, 51, EC]
    EM = np.empty((E, 51), F32)
    EM[:, 0:33] = af[j_idx, 100:133]
    EM[:, 33:47] = ef
    EM[:, 47] = atom_type[i_idx]
    EM[:, 48] = atom_type[j_idx]
    EM[:, 49] = d_edge
    EM[:, 50] = env_edge
    erows = np.zeros((NCORES, 51, EC), F32)
    erows[:, 49, :] = 0.5
    erows[oc, :, lr] = EM

    # --- triplets: sort by (core, window), rank within window
    e_t = idx_kj
    ct = oc[e_t]
    wl_t = lr[e_t] // 128
    sr_t = (lr[e_t] % 128).astype(F32)
    gid = ct * NW + wl_t
    t_ord = np.argsort(gid, kind="stable")
    gids = gid[t_ord]
    starts = np.searchsorted(gids, np.arange(NCORES * NW))
    pos = np.arange(T) - starts[gids]
    assert pos.max() < TPW

    angle_arr = np.zeros((NCORES, NW * TPW), F32)
    seg_arr = np.full((NCORES, NW * TPW), -1.0, F32)
    addr = wl_t[t_ord] * TPW + pos
    angle_arr[ct[t_ord], addr] = angle[t_ord]
    seg_arr[ct[t_ord], addr] = sr_t[t_ord]

    TS = 4 * TPW
    tripc = np.stack([angle_arr.reshape(NCORES, NSC, TS),
                      seg_arr.reshape(NCORES, NSC, TS)], axis=2)  # [8,NSC,2,TS]
    tripc = tripc.reshape(NCORES, 2 * K_FIX, EC)     # NW*2*TPW == 2*K_FIX*EC
    segcol = seg_arr.reshape(NCORES, NW, K_FIX, 128).transpose(0, 3, 1, 2) \
                    .reshape(NCORES, 128, NW * K_FIX)

    big = np.concatenate([erows, tripc], axis=1)     # [8, 51+2K, EC]

    # --- income bonds by (owner core, atom window)
    EB = ib_eid.shape[0]
    bo = oc[ib_eid]
    aw = ib_atom // 128
    key2 = bo * NAW + aw
    cnt2 = np.bincount(key2, minlength=NCORES * NAW)
    K_A = max(1, -(-int(cnt2.max()) // 128))
    b_ord = np.argsort(key2, kind="stable")
    k2s = key2[b_ord]
    starts2 = np.searchsorted(k2s, np.arange(NCORES * NAW))
    pos2 = np.arange(EB) - starts2[k2s]
    kk = pos2 // 128
    pp = pos2 % 128
    srcr = np.zeros((NCORES, 128, NAW * K_A), I32)
    tgt = np.full((NCORES, 128, NAW * K_A), -1.0, F32)
    srcr[bo[b_ord], pp, aw[b_ord] * K_A + kk] = lr[ib_eid[b_ord]].astype(I32)
    tgt[bo[b_ord], pp, aw[b_ord] * K_A + kk] = (ib_atom[b_ord] % 128).astype(F32)

    colc = np.concatenate([segcol, tgt], axis=2)     # [8, 128, NW*K + NAW*K_A]

    # --- atom shard rows for the output head
    NA = NAW * 128
    ASH = NA // NCORES
    ash = np.zeros((NCORES, 34, ASH), F32)
    at_pad = np.zeros(NA, F32)
    at_pad[:N] = atom_type
    af33_pad = np.zeros((NA, 33), F32)
    af33_pad[:N] = af[:, 100:133]
    ash[:, 0:33, :] = af33_pad.reshape(NCORES, ASH, 33).transpose(0, 2, 1)
    ash[:, 33, :] = at_pad.reshape(NCORES, ASH)

    P = dict(N=N, E=E, T=T, FA=FA, NW=NW, NSC=NSC, K_FIX=K_FIX, TPW=TPW,
             EC=EC, NAW=NAW, K_A=K_A, ASH=ASH)
    arrays = dict(
        big=big.reshape(NCORES * (51 + 2 * K_FIX), EC),
        colc=colc.reshape(NCORES * 128, NW * K_FIX + NAW * K_A),
        srcr=srcr.reshape(NCORES * 128, NAW * K_A),
        ash=ash.reshape(NCORES * 34, ASH),
    )
    return P, arrays


# ----------------------------------------------------------------------------
# replicated constants blob [128, CW]
# ----------------------------------------------------------------------------

def _const_layout():
    """Column layouts: rows tensor [1, RW] (all partition-0 row constants)
    and constf blob [128, CW]."""
    rows = dict(ones=(0, 512), freqn=(512, NR), svecn=(528, NS), q025=(534, NS))
    c = 544
    for l in range(L):
        rows[f"bdownr4_{l}"] = (c, 512); c += 512
        rows[f"bupr4_{l}"] = (c, 512); c += 512
    for l in range(L):
        rows[f"b_rbf2r_{l}"] = (c, 128); c += 128
    RW = c

    wnames = ["W1r", "W1a33", "W1e", "TI", "TJ", "Wrbf", "Wemb_r",
              "Wo", "Wo33", "Wom", "identity", "iotam", "REP6", "REPC"]
    for l in range(L):
        wnames += [f"Wkj{l}", f"Wrbf2{l}", f"Wsbf1{l}", f"Wsbf2{l}",
                   f"Wdown{l}", f"Wup{l}", f"Wres1{l}", f"Wres2{l}"]
    blocks = {n: i * 128 for i, n in enumerate(wnames)}
    bc0 = len(wnames) * 128
    bnames = ["b_i1", "b_rbf", "b_emb", "b_o", "iotac"] + \
             [f"b_kj{l}" for l in range(L)] + \
             [f"b_res1_{l}" for l in range(L)] + [f"b_res2_{l}" for l in range(L)]
    bias_cols = {n: bc0 + i for i, n in enumerate(bnames)}
    CW = bc0 + len(bnames)
    return RW, rows, CW, blocks, bias_cols


def _build_constf(inputs):
    W = {k: np.asarray(v, F32) for k, v in inputs.items()}
    RW, rows, CW, blocks, bias_cols = _const_layout()

    rw = np.zeros((1, RW), F32)

    def putr(name, vec):
        c, n = rows[name]
        rw[0, c:c + len(vec)] = np.asarray(vec, F32)

    putr("ones", np.ones(512, F32))
    putr("freqn", W["bessel_freq"] / TWO_PI)
    putr("svecn", np.arange(NS, dtype=F32) / TWO_PI)
    putr("q025", np.full(NS, 0.25, F32))
    for l in range(L):
        putr(f"bdownr4_{l}", np.tile(W["L_down_b"][l], 4))
        putr(f"bupr4_{l}", np.tile(W["L_up_b"][l], 4))
        putr(f"b_rbf2r_{l}", W["L_rbf2_b"][l])

    cf = np.zeros((128, CW), F32)

    def put(name, arr):
        c = blocks[name]
        a = np.asarray(arr, F32)
        cf[:a.shape[0], c:c + a.shape[1]] = a

    put("W1r", W["W_i1_w"][:100])
    put("W1a33", W["W_i1_w"][100:133])
    put("W1e", W["W_i1_w"][133:147])
    put("TI", W["emb_table"] @ W["lin_emb_w"][:H])
    put("TJ", W["emb_table"] @ W["lin_emb_w"][H:2 * H])
    put("Wrbf", W["lin_rbf_w"])
    put("Wemb_r", W["lin_emb_w"][2 * H:])
    put("Wo", W["W_o_w"][:100])
    put("Wo33", W["W_o_w"][100:133])
    put("Wom", W["W_o_w"][133:])
    put("identity", np.eye(128, dtype=F32))
    put("iotam", np.tile(np.arange(128, dtype=F32), (128, 1)))
    put("REP6", np.tile(np.eye(NR, dtype=F32), (1, NS)))
    put("REPC", np.repeat(np.eye(NS, dtype=F32), NR, axis=1))
    for l in range(L):
        put(f"Wkj{l}", W["L_kj_w"][l])
        put(f"Wrbf2{l}", W["L_rbf2_w"][l])
        put(f"Wsbf1{l}", W["L_sbf1_w"][l])
        put(f"Wsbf2{l}", W["L_sbf2_w"][l])
        put(f"Wdown{l}", W["L_down_w"][l])
        put(f"Wup{l}", W["L_up_w"][l])
        put(f"Wres1{l}", W["L_res1_w"][l])
        put(f"Wres2{l}", W["L_res2_w"][l])

    def putb(name, vec):
        cf[:, bias_cols[name]] = np.asarray(vec, F32)

    putb("b_i1", W["W_i1_b"])
    putb("b_rbf", W["lin_rbf_b"])
    putb("b_emb", W["lin_emb_b"])
    putb("b_o", W["W_o_b"])
    putb("iotac", np.arange(128, dtype=F32))
    for l in range(L):
        putb(f"b_kj{l}", W["L_kj_b"][l])
        putb(f"b_res1_{l}", W["L_res1_b"][l])
        putb(f"b_res2_{l}", W["L_res2_b"][l])
    return rw, cf


# ----------------------------------------------------------------------------
# bass program
# ----------------------------------------------------------------------------

def _build_nc(P):
    import concourse.bass as bass
    import concourse.bacc as bacc
    import concourse.mybir as mybir
    import concourse.tile as tile
    from concourse.bass import IndirectOffsetOnAxis

    DT = mybir.dt.float32
    DI = mybir.dt.int32
    NW, NSC, K, TPW = P["NW"], P["NSC"], P["K_FIX"], P["TPW"]
    EC, NAW, K_A, ASH = P["EC"], P["NAW"], P["K_A"], P["ASH"]
    TS = 4 * TPW
    NQ = TS // 512
    NWK = NW * K
    NAKA = NAW * K_A
    RW, ROWS, CW, BLK, BCOL = _const_layout()

    nc = bacc.Bacc("TRN2", target_bir_lowering=False, debug=False,
                   num_devices=NCORES)

    d_big = nc.dram_tensor("big", [51 + 2 * K, EC], DT, kind="ExternalInput")
    d_colc = nc.dram_tensor("colc", [128, NWK + NAKA], DT, kind="ExternalInput")
    d_srcr = nc.dram_tensor("srcr", [128, NAKA], DI, kind="ExternalInput")
    d_ash = nc.dram_tensor("ash", [34, ASH], DT, kind="ExternalInput")
    d_rows = nc.dram_tensor("rowsc", [1, RW], DT, kind="ExternalInput")
    d_cf = nc.dram_tensor("constf", [128, CW], DT, kind="ExternalInput")
    d_out = nc.dram_tensor("outT", [H, ASH], DT, kind="ExternalOutput")

    RELU = mybir.ActivationFunctionType.Relu
    SIN = mybir.ActivationFunctionType.Sin
    ADD = mybir.AluOpType.add
    MULT = mybir.AluOpType.mult
    ISEQ = mybir.AluOpType.is_equal
    MAX = mybir.AluOpType.max

    # window/psum-quadrant segments for per-window matmuls over TS cols
    segs = []
    for q in range(NQ):
        s0 = q * 512
        while s0 < (q + 1) * 512:
            wi = s0 // TPW
            end = min((wi + 1) * TPW, (q + 1) * 512)
            segs.append((q, s0 - q * 512, wi, s0, end - s0))
            s0 = end

    with tile.TileContext(nc) as tc:
        with (
            tc.tile_pool(name="const", bufs=1) as cpool,
            tc.tile_pool(name="sbw", bufs=1) as sbw,
            tc.tile_pool(name="sbm", bufs=2) as sbm,
            tc.tile_pool(name="sb03", bufs=1) as sb03,
            tc.tile_pool(name="sbs", bufs=1) as sbs,
            tc.tile_pool(name="ps", bufs=5, space="PSUM") as ps,
            tc.tile_pool(name="pagg", bufs=2, space="PSUM") as pag,
            tc.tile_pool(name="drm", bufs=1, space="DRAM") as dram,
        ):
            CF = cpool.tile([128, CW], DT, tag="CF")
            nc.sync.dma_start(CF[:], d_cf[:])
            RD = cpool.tile([1, RW], DT, tag="RD")
            nc.sync.dma_start(RD[:], d_rows[:])
            CC = cpool.tile([128, NWK + NAKA], DT, tag="CC")
            nc.sync.dma_start(CC[:], d_colc[:])
            SRT = cpool.tile([128, NAKA], DI, tag="SRT")
            nc.sync.dma_start(SRT[:], d_srcr[:])
            ZER = cpool.tile([128, 512], DT, tag="ZER")
            nc.vector.memset(ZER[:], 0.0)

            def Wb(name, rows=128):
                c = BLK[name]
                return CF[0:rows, c:c + 128]

            def Bc(name):
                c = BCOL[name]
                return CF[:, c:c + 1]

            def Rw(name, n=None):
                c, w = ROWS[name]
                return RD[:, c:c + (w if n is None else n)]

            ONES = RD[:, 0:512]        # ones row; slice cols as needed
            FREQN = Rw("freqn")
            SVECN = Rw("svecn")
            Q025 = Rw("q025")
            IOTAC = Bc("iotac")
            IDENT = Wb("identity")
            IOTAM = Wb("iotam")

            msg = [dram.tile([H, EC], DT, tag="msgA", name="msgA"),
                   dram.tile([H, EC], DT, tag="msgB", name="msgB")]
            rbfeT = dram.tile([H, EC], DT, tag="rbfeT")
            rbf0T = dram.tile([EC, NR], DT, tag="rbf0T")
            msgRM = dram.tile([EC, H], DT, tag="msgRM")
            apart = dram.tile([NCORES, H, ASH], DT, tag="apart")
            asum = dram.tile([H, ASH], DT, tag="asum")

            def onehot(row_ap, tagb, tag):
                """[128,512] one-hot: out[p,e] = (row[e] == p)."""
                raw = sb03.tile([128, 512], DT, tag="ohraw", name="ohraw")
                nc.gpsimd.partition_broadcast(raw[:], row_ap)
                oh = sb03.tile([128, 512], DT, tag=tag, name="oh")
                nc.vector.tensor_scalar(oh[:], raw[:], IOTAC, None, ISEQ)
                return oh

            # ---------------- phase 0: embedding ----------------
            for sc in range(NSC):
                cs = slice(sc * 512, sc * 512 + 512)
                erA = sb03.tile([33, 512], DT, tag="erA")
                nc.sync.dma_start(erA[:], d_big[0:33, cs])
                erB = sbs.tile([14, 512], DT, tag="erB")
                nc.sync.dma_start(erB[:], d_big[33:47, cs])
                erTi = sbs.tile([1, 512], DT, tag="erTi")
                nc.sync.dma_start(erTi[:], d_big[47:48, cs])
                erTj = sbs.tile([1, 512], DT, tag="erTj")
                nc.sync.dma_start(erTj[:], d_big[48:49, cs])
                erD = sbs.tile([1, 512], DT, tag="erD")
                nc.sync.dma_start(erD[:], d_big[49:50, cs])
                erE = sbs.tile([1, 512], DT, tag="erE")
                nc.sync.dma_start(erE[:], d_big[50:51, cs])
                ohj = onehot(erTj[:], "ohjb", "ohj")
                ohi = onehot(erTi[:], "ohib", "ohi")

                pm = ps.tile([128, 512], DT, tag="P")
                nc.tensor.matmul(pm[:], Wb("W1r"), ohj[:], start=True, stop=False)
                nc.tensor.matmul(pm[:], Wb("W1a33", 33), erA[:], start=False, stop=False)
                nc.tensor.matmul(pm[:], Wb("W1e", 14), erB[:], start=False, stop=True)
                m0 = sb03.tile([128, 512], DT, tag="m0")
                nc.vector.scalar_tensor_tensor(m0[:], pm[:], Bc("b_i1"), ZER[:], ADD, MAX)
                nc.sync.dma_start(msg[0][:, cs], m0[:])

                parg = ps.tile([NR, 512], DT, tag="P")
                nc.tensor.matmul(parg[:], FREQN, erD[:], start=True, stop=True)
                qi = sbs.tile([NR, 512], DI, tag="qi16")
                nc.vector.tensor_copy(qi[:], parg[:])
                qf = sbs.tile([NR, 512], DT, tag="qf16")
                nc.vector.tensor_copy(qf[:], qi[:])
                y = sbs.tile([NR, 512], DT, tag="y16")
                nc.vector.scalar_tensor_tensor(y[:], qf[:], -1.0, parg[:], MULT, ADD)
                s = sbs.tile([NR, 512], DT, tag="s16")
                nc.scalar.activation(s[:], y[:], SIN, scale=TWO_PI)
                penv = ps.tile([NR, 512], DT, tag="P")
                nc.tensor.matmul(penv[:], ONES[:, 0:NR], erE[:], start=True, stop=True)
                rbf0 = sbs.tile([NR, 512], DT, tag="rbf0")
                nc.vector.tensor_tensor(rbf0[:], s[:], penv[:], op=MULT)
                nc.sync.dma_start(rbf0T[sc * 512:(sc + 1) * 512, :].transpose([1, 0]),
                                  rbf0[:])

                prh = ps.tile([128, 512], DT, tag="P")
                nc.tensor.matmul(prh[:], Wb("Wrbf", NR), rbf0[:], start=True, stop=True)
                rbfh = sb03.tile([128, 512], DT, tag="rbfh")
                nc.vector.scalar_tensor_tensor(rbfh[:], prh[:], Bc("b_rbf"), ZER[:], ADD, MAX)

                pre = ps.tile([128, 512], DT, tag="P")
                nc.tensor.matmul(pre[:], Wb("TI"), ohi[:], start=True, stop=False)
                nc.tensor.matmul(pre[:], Wb("TJ"), ohj[:], start=False, stop=False)
                nc.tensor.matmul(pre[:], Wb("Wemb_r"), rbfh[:], start=False, stop=True)
                rbe = sb03.tile([128, 512], DT, tag="rbe")
                nc.vector.scalar_tensor_tensor(rbe[:], pre[:], Bc("b_emb"), ZER[:], ADD, MAX)
                nc.sync.dma_start(rbfeT[:, cs], rbe[:])

            # ---------------- phase 1: interaction layers ----------------
            trip_flat = d_big[51:51 + 2 * K, :].rearrange("a b -> (a b)")
            for l in range(L):
                src, dst = msg[l % 2], msg[(l + 1) % 2]
                for sc in range(NSC):
                    cs = slice(sc * 512, sc * 512 + 512)
                    mt = sbm.tile([128, 512], DT, tag="mt")
                    nc.sync.dma_start(mt[:], src[:, cs])
                    ret = sbm.tile([128, 512], DT, tag="ret")
                    nc.sync.dma_start(ret[:], rbfeT[:, cs])

                    pkj = ps.tile([128, 512], DT, tag="P")
                    nc.tensor.matmul(pkj[:], Wb(f"Wkj{l}"), mt[:], start=True, stop=True)
                    kj = sbm.tile([128, 512], DT, tag="kj")
                    nc.vector.scalar_tensor_tensor(kj[:], pkj[:], Bc(f"b_kj{l}"), ZER[:], ADD, MAX)
                    pr = ps.tile([128, 512], DT, tag="P")
                    nc.tensor.matmul(pr[:], Rw(f"b_rbf2r_{l}"), ONES[:, 0:512],
                                     start=True, stop=False)
                    nc.tensor.matmul(pr[:], Wb(f"Wrbf2{l}"), ret[:], start=False, stop=True)
                    xkr = sbm.tile([128, 512], DT, tag="xkr")
                    nc.vector.scalar_tensor_tensor(xkr[:], pr[:], 0.0, kj[:], MAX, MULT)

                    # down-projection for 4 windows: y4[e, (wi,h')]
                    pd = ps.tile([128, 512], DT, tag="P")
                    nc.tensor.matmul(pd[:], ONES[:, 0:128], Rw(f"bdownr4_{l}"),
                                     start=True, stop=False)
                    for wi in range(4):
                        ws = slice(wi * 128, wi * 128 + 128)
                        nc.tensor.matmul(pd[:, ws], xkr[:, ws], Wb(f"Wdown{l}"),
                                         start=False, stop=(wi == 3))
                    y4 = sbm.tile([128, 512], DT, tag="y4")
                    nc.vector.tensor_scalar_max(y4[:], pd[:], 0.0)

                    # triplet metadata (separate base-0 row tiles)
                    trowA = sbs.tile([1, TS], DT, tag="trow")
                    nc.sync.dma_start(
                        trowA[:],
                        trip_flat[sc * 2 * TS:sc * 2 * TS + TS].unsqueeze(0))
                    trowS = sbs.tile([1, TS], DT, tag="trow")
                    nc.sync.dma_start(
                        trowS[:],
                        trip_flat[sc * 2 * TS + TS:(sc + 1) * 2 * TS].unsqueeze(0))
                    segb = sbw.tile([128, TS], DT, tag="segb")
                    nc.gpsimd.partition_broadcast(segb[:], trowS[:])
                    esub = sbw.tile([128, TS], DT, tag="esub")
                    nc.vector.tensor_scalar(esub[:], segb[:], IOTAC, None, ISEQ)

                    # cbf [6, TS]
                    cbf = sbw.tile([NS, TS], DT, tag="cbf")
                    for q in range(NQ):
                        qs = slice(q * 512, q * 512 + 512)
                        pa = ps.tile([NS, 512], DT, tag="P")
                        nc.tensor.matmul(pa[:], Q025, ONES[:, 0:512], start=True, stop=False)
                        nc.tensor.matmul(pa[:], SVECN, trowA[:, qs], start=False, stop=True)
                        qi6 = sbs.tile([NS, 512], DI, tag="qi6")
                        nc.vector.tensor_copy(qi6[:], pa[:])
                        qf6 = sbs.tile([NS, 512], DT, tag="qf6")
                        nc.vector.tensor_copy(qf6[:], qi6[:])
                        y6 = sbs.tile([NS, 512], DT, tag="y6")
                        nc.vector.scalar_tensor_tensor(y6[:], qf6[:], -1.0, pa[:], MULT, ADD)
                        nc.scalar.activation(cbf[:, qs], y6[:], SIN, scale=TWO_PI)

                    # per-triplet rbf [16, TS] via expand of per-edge rbf0
                    rbfe4 = sbs.tile([128, 4 * NR], DT, tag="rbfe4")
                    for wi in range(4):
                        r0 = (sc * 4 + wi) * 128
                        nc.sync.dma_start(rbfe4[:, wi * NR:(wi + 1) * NR],
                                          rbf0T[r0:r0 + 128, :])
                    rbf_s = sbw.tile([NR, TS], DT, tag="rbf_s")
                    for (q, c0, wi, t0, ln) in segs:
                        prb = ps.tile([NR, 512], DT, tag="P")
                        nc.tensor.matmul(prb[:, 0:ln],
                                         rbfe4[:, wi * NR:(wi + 1) * NR],
                                         esub[:, t0:t0 + ln], start=True, stop=True)
                        nc.scalar.copy(rbf_s[:, t0:t0 + ln], prb[:, 0:ln])

                    # sbf [96, TS]: (REP6 @ rbf) * (REPC @ cbf)
                    sbf = sbw.tile([NS * NR, TS], DT, tag="sbf")
                    for q in range(NQ):
                        qs = slice(q * 512, q * 512 + 512)
                        pr96 = ps.tile([NS * NR, 512], DT, tag="P")
                        nc.tensor.matmul(pr96[:], Wb("REP6", NR)[:, 0:NS * NR],
                                         rbf_s[:, qs], start=True, stop=True)
                        pc96 = ps.tile([NS * NR, 512], DT, tag="P")
                        nc.tensor.matmul(pc96[:], Wb("REPC", NS)[:, 0:NS * NR],
                                         cbf[:, qs], start=True, stop=True)
                        c96 = sbs.tile([NS * NR, 512], DT, tag="c96")
                        nc.scalar.copy(c96[:], pc96[:])
                        nc.vector.tensor_tensor(sbf[:, qs], pr96[:], c96[:], op=MULT)

                    # s2 = relu(Wsbf2 relu(Wsbf1 sbf))
                    s2w = sbw.tile([128, TS], DT, tag="segb")
                    for q in range(NQ):
                        qs = slice(q * 512, q * 512 + 512)
                        ps1 = ps.tile([128, 512], DT, tag="P")
                        nc.tensor.matmul(ps1[:], Wb(f"Wsbf1{l}", NS * NR), sbf[:, qs],
                                         start=True, stop=True)
                        s1 = sbs.tile([128, 512], DT, tag="s1")
                        nc.vector.tensor_scalar_max(s1[:], ps1[:], 0.0)
                        ps2 = ps.tile([128, 512], DT, tag="P")
                        nc.tensor.matmul(ps2[:], Wb(f"Wsbf2{l}"), s1[:], start=True, stop=True)
                        nc.vector.tensor_scalar_max(s2w[:, qs], ps2[:], 0.0)

                    # expand y4 to triplets, multiply by s2
                    xsw = sbw.tile([128, TS], DT, tag="xsw")
                    pxq = [None] * NQ
                    for (q, c0, wi, t0, ln) in segs:
                        if pxq[q] is None:
                            pxq[q] = ps.tile([128, 512], DT, tag="P", name=f"px{q}")
                        nc.tensor.matmul(pxq[q][:, c0:c0 + ln],
                                         y4[:, wi * 128:(wi + 1) * 128],
                                         esub[:, t0:t0 + ln], start=True, stop=True)
                    for q in range(NQ):
                        qs = slice(q * 512, q * 512 + 512)
                        nc.vector.tensor_tensor(xsw[:, qs], pxq[q][:], s2w[:, qs], op=MULT)

                    # up-projection (t-major) with bias, relu
                    zw = sbw.tile([128, TS], DT, tag="xsw")
                    for q in range(NQ):
                        pz = ps.tile([128, 512], DT, tag="P")
                        nc.tensor.matmul(pz[:], ONES[:, 0:128], Rw(f"bupr4_{l}"),
                                         start=True, stop=False)
                        for c in range(4):
                            ch = q * 4 + c
                            nc.tensor.matmul(pz[:, c * 128:(c + 1) * 128],
                                             xsw[:, ch * 128:(ch + 1) * 128],
                                             Wb(f"Wup{l}"), start=False, stop=(c == 3))
                        nc.vector.tensor_scalar_max(zw[:, q * 512:(q + 1) * 512], pz[:], 0.0)

                    # scatter one-hots for all 4K chunks at once
                    ssw = sbw.tile([128, 4 * K * 128], DT, tag="esub")
                    nc.vector.tensor_tensor(
                        ssw[:].rearrange("p (c e) -> p c e", c=4 * K),
                        IOTAM.unsqueeze(1).to_broadcast([128, 4 * K, 128]),
                        CC[:, sc * 4 * K:(sc + 1) * 4 * K]
                        .unsqueeze(2).to_broadcast([128, 4 * K, 128]),
                        op=ISEQ)

                    # scatter-add into per-window [h, e] then residual block
                    aggw = sbm.tile([128, 512], DT, tag="aggw")
                    for wi in range(4):
                        pga = pag.tile([128, 128], DT, tag="A")
                        for k in range(K):
                            ch = wi * K + k
                            nc.tensor.matmul(pga[:],
                                             zw[:, ch * 128:(ch + 1) * 128],
                                             ssw[:, ch * 128:(ch + 1) * 128],
                                             start=(k == 0), stop=(k == K - 1))
                        nc.scalar.copy(aggw[:, wi * 128:(wi + 1) * 128], pga[:])

                    p1 = ps.tile([128, 512], DT, tag="P")
                    nc.tensor.matmul(p1[:], Wb(f"Wres1{l}"), aggw[:], start=True, stop=True)
                    r1 = sbm.tile([128, 512], DT, tag="r1")
                    nc.vector.scalar_tensor_tensor(r1[:], p1[:], Bc(f"b_res1_{l}"), ZER[:], ADD, MAX)
                    p2 = ps.tile([128, 512], DT, tag="P")
                    nc.tensor.matmul(p2[:], Wb(f"Wres2{l}"), r1[:], start=True, stop=True)
                    r2 = sbm.tile([128, 512], DT, tag="r2")
                    nc.vector.scalar_tensor_tensor(r2[:], p2[:], Bc(f"b_res2_{l}"), ZER[:], ADD, MAX)
                    mnew = sbm.tile([128, 512], DT, tag="mnew")
                    nc.vector.tensor_tensor(mnew[:], aggw[:], r2[:], op=ADD)
                    nc.vector.tensor_tensor(mnew[:], mnew[:], mt[:], op=ADD)
                    nc.sync.dma_start(dst[:, cs], mnew[:])

                    if l == L - 1:
                        mrm = sbm.tile([128, 512], DT, tag="mrm")
                        for wi in range(4):
                            pt = pag.tile([128, 128], DT, tag="A")
                            nc.tensor.transpose(pt[:], mnew[:, wi * 128:(wi + 1) * 128],
                                                IDENT)
                            nc.scalar.copy(mrm[:, wi * 128:(wi + 1) * 128], pt[:])
                        for wi in range(4):
                            r0 = (sc * 4 + wi) * 128
                            nc.sync.dma_start(msgRM[r0:r0 + 128, :],
                                              mrm[:, wi * 128:(wi + 1) * 128])

            # ---------------- phase 2: atom aggregation ----------------
            WPB = NAW // NCORES
            for g in range(NAW // 4):
                sat4 = sbs.tile([128, 4 * K_A * 128], DT, tag="sat4")
                nc.vector.tensor_tensor(
                    sat4[:].rearrange("p (c e) -> p c e", c=4 * K_A),
                    IOTAM.unsqueeze(1).to_broadcast([128, 4 * K_A, 128]),
                    CC[:, NWK + g * 4 * K_A: NWK + (g + 1) * 4 * K_A]
                    .unsqueeze(2).to_broadcast([128, 4 * K_A, 128]),
                    op=ISEQ)
                apw = sbs.tile([128, 512], DT, tag="apw")
                for wi in range(4):
                    w = g * 4 + wi
                    pap = pag.tile([128, 128], DT, tag="A")
                    for k in range(K_A):
                        gath = sbs.tile([128, 128], DT, tag="gath")
                        nc.gpsimd.indirect_dma_start(
                            out=gath[:], out_offset=None,
                            in_=msgRM[:],
                            in_offset=IndirectOffsetOnAxis(
                                ap=SRT[:, w * K_A + k:w * K_A + k + 1], axis=0))
                        nc.tensor.matmul(pap[:], gath[:],
                                         sat4[:, (wi * K_A + k) * 128:(wi * K_A + k + 1) * 128],
                                         start=(k == 0), stop=(k == K_A - 1))
                    nc.scalar.copy(apw[:, wi * 128:(wi + 1) * 128], pap[:])
                blk = (g * 4) // WPB
                col = ((g * 4) % WPB) * 128
                nc.sync.dma_start(apart[blk, :, col:col + 512], apw[:])

            nc.gpsimd.collective_compute(
                "ReduceScatter", ADD,
                replica_groups=[list(range(NCORES))],
                ins=[apart.opt()], outs=[asum.opt()])

            # ---------------- phase 3: output head ----------------
            for jb in range(ASH // 512):
                cs = slice(jb * 512, jb * 512 + 512)
                ash33 = sb03.tile([33, 512], DT, tag="ash33")
                nc.sync.dma_start(ash33[:], d_ash[0:33, cs])
                asht = sbs.tile([1, 512], DT, tag="asht")
                nc.sync.dma_start(asht[:], d_ash[33:34, cs])
                ams = sb03.tile([128, 512], DT, tag="ams")
                nc.sync.dma_start(ams[:], asum[:, cs])
                oha = onehot(asht[:], "ohab", "oha")
                po = ps.tile([128, 512], DT, tag="P")
                nc.tensor.matmul(po[:], Wb("Wo"), oha[:], start=True, stop=False)
                nc.tensor.matmul(po[:], Wb("Wo33", 33), ash33[:], start=False, stop=False)
                nc.tensor.matmul(po[:], Wb("Wom"), ams[:], start=False, stop=True)
                ot = sb03.tile([128, 512], DT, tag="ot")
                nc.vector.scalar_tensor_tensor(ot[:], po[:], Bc("b_o"), ZER[:], ADD, MAX)
                nc.sync.dma_start(d_out[:, cs], ot[:])

    nc.compile()
    return nc


# ----------------------------------------------------------------------------
# execution via PJRT (AOT-compiled, serialized to /tmp for warm starts)
# ----------------------------------------------------------------------------

def _exe_meta_from_nc(nc):
    import concourse.mybir as mybir
    partition_name = nc.partition_id_tensor.name if nc.partition_id_tensor else None
    in_names, out_names, out_shapes, out_dtypes = [], [], [], []
    for alloc in nc.m.functions[0].allocations:
        if not isinstance(alloc, mybir.MemoryLocationSet):
            continue
        name = alloc.memorylocations[0].name
        if alloc.kind == "ExternalInput":
            if name != partition_name:
                in_names.append(name)
        elif alloc.kind == "ExternalOutput":
            out_names.append(name)
            out_shapes.append(tuple(alloc.tensor_shape))
            out_dtypes.append(np.dtype(mybir.dt.np(alloc.dtype)).name)
    return dict(partition_name=partition_name, in_names=in_names,
                out_names=out_names, out_shapes=out_shapes, out_dtypes=out_dtypes)


def _compile_exe(nc, meta, arrays):
    import jax
    from jax.sharding import Mesh, PartitionSpec
    from jax.experimental.shard_map import shard_map
    import concourse.bass2jax as b2j

    b2j.install_neuronx_cc_hook()
    partition_name = meta["partition_name"]
    in_names = meta["in_names"]
    out_names = meta["out_names"]
    out_avals = [jax.core.ShapedArray(s, np.dtype(d))
                 for s, d in zip(meta["out_shapes"], meta["out_dtypes"])]
    n_params = len(in_names)
    n_outs = len(out_names)
    in_names_full = in_names + out_names + ([partition_name] if partition_name else [])

    def _body(*args):
        operands = list(args)
        if partition_name is not None:
            operands.append(b2j.partition_id_tensor())
        outs = b2j._bass_exec_p.bind(
            *operands, out_avals=tuple(out_avals), in_names=tuple(in_names_full),
            out_names=tuple(out_names), lowering_input_output_aliases=(),
            sim_require_finite=True, sim_require_nnan=True, nc=nc)
        return tuple(outs)

    devices = jax.devices()[:NCORES]
    mesh = Mesh(np.asarray(devices), ("core",))
    sharded = jax.jit(shard_map(_body, mesh=mesh,
                                in_specs=(PartitionSpec("core"),) * (n_params + n_outs),
                                out_specs=(PartitionSpec("core"),) * n_outs,
                                check_rep=False),
                      donate_argnums=tuple(range(n_params, n_params + n_outs)),
                      keep_unused=True)
    avals = [jax.ShapeDtypeStruct(arrays[n].shape, arrays[n].dtype) for n in in_names]
    zavals = [jax.ShapeDtypeStruct((NCORES * s[0],) + tuple(s[1:]), np.dtype(d))
              for s, d in zip(meta["out_shapes"], meta["out_dtypes"])]
    compiled = sharded.lower(*avals, *zavals).compile()
    return compiled


def kernel(**inputs):
    global LAST_RESULTS
    LAST_RESULTS = None
    import time as _time
    _t0 = _time.time()
    _tick = lambda tag: print(f"[kernel-timing] {tag}: {_time.time() - _t0:.2f}s",
                              file=sys.stderr, flush=True)
    import jax
    try:
        jax.config.update("jax_compilation_cache_dir", "/tmp/jax_cc_cache")
        jax.config.update("jax_persistent_cache_min_entry_size_bytes", -1)
        jax.config.update("jax_persistent_cache_min_compile_time_secs", 0.0)
    except Exception:
        pass
    from jax.sharding import Mesh, PartitionSpec, NamedSharding
    import jax.experimental.serialize_executable as se

    P, arrays = _pack(inputs)
    rw, cf = _build_constf(inputs)
    arrays["rowsc"] = np.tile(rw, (NCORES, 1))
    arrays["constf"] = np.tile(cf, (NCORES, 1))
    _tick("pack")

    key = hashlib.sha256(
        (VERSION + repr(sorted(P.items()))).encode()).hexdigest()[:16]
    cache_base = os.path.join(CACHE_DIR, key)

    devices = jax.devices()[:NCORES]
    mesh = Mesh(np.asarray(devices), ("core",))
    shin = NamedSharding(mesh, PartitionSpec("core"))

    compiled = meta = None
    if key in _PROC_MEMO:
        compiled, meta = _PROC_MEMO[key]

    # start async uploads before any compile work
    dev = {n: jax.device_put(a, shin) for n, a in arrays.items()}
    _tick("device_put dispatched")

    if compiled is None:
        try:
            with open(cache_base + ".meta.pkl", "rb") as f:
                meta = pickle.load(f)
            with open(cache_base + ".exe.bin", "rb") as f:
                d = pickle.load(f)
            compiled = se.deserialize_and_load(d["payload"], d["in_tree"], d["out_tree"])
            _tick("exe deserialized (warm)")
        except Exception:
            compiled = meta = None

    if compiled is None:
        nc = _build_nc(P)
        _tick("bass program built")
        meta = _exe_meta_from_nc(nc)
        compiled = _compile_exe(nc, meta, arrays)
        _tick("AOT compile done")
        try:
            os.makedirs(CACHE_DIR, exist_ok=True)
            payload, in_tree, out_tree = se.serialize(compiled)
            fd, tmp = tempfile.mkstemp(dir=CACHE_DIR)
            with os.fdopen(fd, "wb") as f:
                pickle.dump(dict(payload=payload, in_tree=in_tree, out_tree=out_tree), f)
            os.replace(tmp, cache_base + ".exe.bin")
            fd, tmp = tempfile.mkstemp(dir=CACHE_DIR)
            with os.fdopen(fd, "wb") as f:
                pickle.dump(meta, f)
            os.replace(tmp, cache_base + ".meta.pkl")
        except Exception:
            pass

    _PROC_MEMO[key] = (compiled, meta)

    zeros = [np.zeros((NCORES * s[0],) + tuple(s[1:]), np.dtype(d))
             for s, d in zip(meta["out_shapes"], meta["out_dtypes"])]
    args = [dev[n] for n in meta["in_names"]] + \
           [jax.device_put(z, shin) for z in zeros]
    outs = compiled(*args)
    out_np = {n: np.asarray(o) for n, o in zip(meta["out_names"], outs)}
    _tick("exec+download done")

    N, ASH = P["N"], P["ASH"]
    outT = out_np["outT"].reshape(NCORES, H, ASH)
    out = np.zeros((N, H), F32)
    for c in range(NCORES):
        lo = c * ASH
        hi = min(N, lo + ASH)
        if hi > lo:
            out[lo:hi] = outT[c][:, :hi - lo].T
    return out
